# revision 1
# baseline (speedup 1.0000x reference)
"""Trainium2 Bass kernel for the gnn_message_passing problem.

Math refactor: the reference computes
    kernel[z,i,j] = einsum('zk,kij->zij', Rk*Yk, cg) * nc0[i,j]
with Rk = R @ rf_mix.T (rank 6) and Yk = Y.T @ ylm_mix.T (rank 9).
Since Rk*Yk has rank<=54 over k, fold the K=1024 contraction into a
precomputed M[p*9+l, ij] = sum_k rf[k,p]*ylm_s[k,l]*cg[k,ij] * nc0[ij]
(computed on device from the cg/rf/ylm/norm inputs), and per point only
contract B[z, pl] = R[z,p]*Y'[z,l] against M - a k=54 matmul. This cuts
compute ~20x and makes the kernel memory-bound (410 MB output).

Distribution: data-parallel over z across 8 NeuronCores; constants
replicated. Full inputs in, full output out.

Precision: the fast fp32r matmul path rounds inputs to an 11-bit
mantissa, so the main contraction uses a hi/lo split:
    out = [Bh;Bl] @ [Mh;Mh]  (k=108, accumulating)  +  Bh @ Ml  (k=54)
which drops only the Bl@Ml term (~2^-24 relative) - fp32-class accuracy
at 1 cycle/row. Everything feeding B (monomials, radial MLP) runs in
exact fp32 (PE fp32 mode, DVE reciprocal, ACT sqrt + one Newton step).
"""

import numpy as np

import concourse.bass as bass
import concourse.tile as tile
from concourse import bacc, mybir
from concourse.bass_utils import run_bass_kernel_spmd

F32 = mybir.dt.float32
F32R = mybir.dt.float32r
ALU = mybir.AluOpType
ACTF = mybir.ActivationFunctionType

# Problem shape (hardcoded per contract)
Z, KDIM, DO, DI, NPATH, H = 100000, 1024, 32, 32, 6, 128
IJ = DO * DI                      # 1024
NCORES = 8
ZC = Z // NCORES                  # 12500 points per core
T = 100                           # point tiles of 128 -> ZC padded to 12800
ZC_PAD = 128 * T
TB = 4                            # tiles per group
NG = T // TB                      # 25 groups
NCH = 10                          # channels: radii, ones, 8 scaled monomials
NKT = KDIM // 128                 # 8 k-tiles for the M build

# Real spherical harmonic constants (l=0,1,2), folded into ylm host-side
C0 = 0.28209479177387814
C1 = 0.4886025119029199
C2A = 1.0925484305920792
C2B = 0.31539156525252005
C2C = 0.5462742152960396
YLM_SCALE = np.array([C0, C1, C1, C1, C2A, C2A, C2B, C2A, C2C], dtype=np.float64)

_CACHE = {}


def _build_program():
    nc = bacc.Bacc("TRN2", target_bir_lowering=False, debug=False,
                   num_devices=NCORES)

    # ---- per-core DRAM I/O ----
    rpl = nc.dram_tensor("rpl", [128, 3 * T], F32, kind="ExternalInput").ap()
    w1e4 = nc.dram_tensor("w1e4", [NCH * TB, TB * 128], F32, kind="ExternalInput").ap()
    ey4 = nc.dram_tensor("ey4", [NCH * TB, TB * 54], F32, kind="ExternalInput").ap()
    w2e = nc.dram_tensor("w2e", [H, 54], F32, kind="ExternalInput").ap()
    b1c = nc.dram_tensor("b1c", [H, 1], F32, kind="ExternalInput").ap()
    b2r = nc.dram_tensor("b2r", [54, 1], F32, kind="ExternalInput").ap()
    cgd = nc.dram_tensor("cgd", [KDIM, IJ], F32, kind="ExternalInput").ap()
    rft = nc.dram_tensor("rft", [128, NKT * NPATH], F32, kind="ExternalInput").ap()
    ylt = nc.dram_tensor("ylt", [128, NKT * 9], F32, kind="ExternalInput").ap()
    ncv = nc.dram_tensor("ncv", [1, IJ], F32, kind="ExternalInput").ap()
    identd = nc.dram_tensor("identd", [128, 128], F32, kind="ExternalInput").ap()
    out = nc.dram_tensor("out", [ZC, IJ], F32, kind="ExternalOutput").ap()

    with tile.TileContext(nc) as tc:
        with tc.tile_pool(name="const", bufs=1) as cpool, \
             tc.tile_pool(name="mbuf", bufs=1) as mpool:
            # ---- resident constants ----
            w1e_sb = cpool.tile([NCH * TB, TB * 128], F32)
            nc.sync.dma_start(w1e_sb[:], w1e4[:])
            ey4_sb = cpool.tile([NCH * TB, TB * 54], F32)
            nc.sync.dma_start(ey4_sb[:], ey4[:])
            w2e_sb = cpool.tile([H, 54], F32)
            nc.sync.dma_start(w2e_sb[:], w2e[:])
            b1_sb = cpool.tile([H, 1], F32)
            nc.sync.dma_start(b1_sb[:], b1c[:])
            b2_sb = cpool.tile([54, 1], F32)
            nc.sync.dma_start(b2_sb[:], b2r[:])
            id_sb = cpool.tile([128, 128], F32)
            nc.sync.dma_start(id_sb[:], identd[:])
            ncv_sb = cpool.tile([1, IJ], F32)
            nc.sync.dma_start(ncv_sb[:], ncv[:])
            ones54 = cpool.tile([1, 54], F32)
            nc.vector.memset(ones54[:], 1.0)

            # M tensors: rows 0-53 = Mh, 54-63 = zeros (partition-alignment
            # filler; engine writes must start at 0/32/64/96), 64-117 = Mh.
            mstk = cpool.tile([118, IJ], F32R)
            ml_sb = cpool.tile([54, IJ], F32R)
            # B hi/lo stack: 3 manually rotated slots along the free dim
            bstk_all = cpool.tile([118, 6 * 128], F32R)
            nc.vector.memset(mstk[32:64, :].bitcast(F32), 0.0)
            nc.vector.memset(bstk_all[32:64, :].bitcast(F32), 0.0)

            # =========================================================
            # Phase 1: build M[pl, ij] from cg, rf, ylm, norm_coef
            # =========================================================
            with tc.tile_pool(name="mpsum", bufs=1, space="PSUM") as mps_pool:
                cg_sb = mpool.tile([128, NKT * IJ], F32)
                cg_r = cgd.rearrange("(kt p) ij -> p kt ij", p=128)
                nc.sync.dma_start(
                    cg_sb[:].rearrange("p (kt ij) -> p kt ij", kt=NKT), cg_r)
                rf_sb = mpool.tile([128, NKT * NPATH], F32)
                nc.sync.dma_start(rf_sb[:], rft[:])
                yl_sb = mpool.tile([128, NKT * 9], F32)
                nc.sync.dma_start(yl_sb[:], ylt[:])

                # W[k, pl] = rf[k,p] * ylm_s[k,l]
                w_sb = mpool.tile([128, NKT * 54], F32)
                for kt in range(NKT):
                    for p in range(NPATH):
                        nc.vector.tensor_scalar(
                            w_sb[:, kt * 54 + p * 9: kt * 54 + p * 9 + 9],
                            yl_sb[:, kt * 9: kt * 9 + 9],
                            rf_sb[:, kt * NPATH + p: kt * NPATH + p + 1],
                            None, ALU.mult)

                m_ps = mps_pool.tile([54, IJ], F32)
                for half in range(2):
                    for kt in range(NKT):
                        nc.tensor.matmul(
                            m_ps[:, half * 512:(half + 1) * 512],
                            w_sb[:, kt * 54:(kt + 1) * 54],
                            cg_sb[:, kt * IJ + half * 512: kt * IJ + half * 512 + 512],
                            start=(kt == 0), stop=(kt == NKT - 1))

                # broadcast norm_coef[...,0] across the 54 partitions
                ncr_ps = mps_pool.tile([54, IJ], F32)
                for half in range(2):
                    nc.tensor.matmul(
                        ncr_ps[:, half * 512:(half + 1) * 512],
                        ones54[:],
                        ncv_sb[:, half * 512:(half + 1) * 512],
                        start=True, stop=True)
                ncr_sb = mpool.tile([54, IJ], F32)
                nc.scalar.copy(ncr_sb[:], ncr_ps[:])

                mf_sb = mpool.tile([54, IJ], F32)
                nc.vector.tensor_tensor(mf_sb[:], m_ps[:], ncr_sb[:], ALU.mult)
                # hi/lo split (fp32r rounding happens on write)
                nc.vector.tensor_copy(mstk[0:54, :], mf_sb[:])
                nc.scalar.copy(mstk[64:118, :], mf_sb[:])
                nc.vector.tensor_tensor(ml_sb[:], mf_sb[:],
                                        mstk[0:54, :].bitcast(F32), ALU.subtract)

            # =========================================================
            # Phase 2: per-point planes [128, T]: radii, ones, monomials
            # =========================================================
            rpl_sb = cpool.tile([128, 3 * T], F32)
            nc.sync.dma_start(rpl_sb[:], rpl[:])
            x = rpl_sb[:, 0:T]
            y = rpl_sb[:, T:2 * T]
            z = rpl_sb[:, 2 * T:3 * T]

            chan = cpool.tile([128, NCH * T], F32)
            aux = cpool.tile([128, 10 * T], F32)

            def ax(i):
                return aux[:, i * T:(i + 1) * T]

            xx, yy, zz, s1, r2, mask, inv2, va, vb, t8 = (ax(i) for i in range(10))
            nc.vector.tensor_tensor(xx, x, x, ALU.mult)
            nc.vector.tensor_tensor(yy, y, y, ALU.mult)
            nc.vector.tensor_tensor(zz, z, z, ALU.mult)
            nc.vector.tensor_tensor(s1, xx, yy, ALU.add)
            nc.vector.tensor_tensor(r2, s1, zz, ALU.add)
            # guard r2 == 0 exactly like the reference's safe_r2
            nc.vector.tensor_scalar(mask, r2, 0.0, None, ALU.is_equal)
            nc.vector.tensor_tensor(mask, r2, mask, ALU.add)     # safe_r2
            nc.vector.reciprocal(inv2, mask)                     # 1/safe_r2 (accurate)
            nc.scalar.sqrt(va, inv2)                             # rsqrt seed ~7e-6
            # one Newton step: v = v*(1.5 - 0.5*safe_r2*v^2)
            nc.vector.tensor_tensor(vb, va, va, ALU.mult)
            nc.vector.tensor_tensor(vb, vb, mask, ALU.mult)
            nc.vector.tensor_scalar(vb, vb, -0.5, 1.5, ALU.mult, ALU.add)
            nc.vector.tensor_tensor(va, va, vb, ALU.mult)        # inv_r

            # chan is stored t-major interleaved (col = t*NCH + c) so each
            # group's transpose input is one contiguous 40-col slice
            chan_v = chan[:].rearrange("p (t c) -> p c t", c=NCH)
            ch = [chan_v[:, i, :] for i in range(NCH)]
            nc.vector.tensor_tensor(ch[0], r2, va, ALU.mult)     # radii
            nc.vector.tensor_scalar(ch[1], r2, 0.0, 1.0, ALU.mult, ALU.add)  # ones
            nc.vector.tensor_tensor(ch[2], y, va, ALU.mult)      # l=1
            nc.vector.tensor_tensor(ch[3], z, va, ALU.mult)      # l=2
            nc.vector.tensor_tensor(ch[4], x, va, ALU.mult)      # l=3
            nc.vector.tensor_tensor(vb, x, y, ALU.mult)
            nc.vector.tensor_tensor(ch[5], vb, inv2, ALU.mult)   # l=4: xy/r2
            nc.vector.tensor_tensor(vb, y, z, ALU.mult)
            nc.vector.tensor_tensor(ch[6], vb, inv2, ALU.mult)   # l=5: yz/r2
            nc.vector.scalar_tensor_tensor(vb, zz, 3.0, r2, ALU.mult, ALU.subtract)
            nc.vector.tensor_tensor(ch[7], vb, inv2, ALU.mult)   # l=6: (3zz-r2)/r2
            nc.vector.tensor_tensor(vb, x, z, ALU.mult)
            nc.vector.tensor_tensor(ch[8], vb, inv2, ALU.mult)   # l=7: xz/r2
            nc.vector.tensor_tensor(t8, xx, yy, ALU.subtract)
            nc.vector.tensor_tensor(ch[9], t8, inv2, ALU.mult)   # l=8: (xx-yy)/r2

            # =========================================================
            # Phase 3: main loop over 25 groups of 4 point-tiles
            # =========================================================
            with tc.tile_pool(name="tps", bufs=2, space="PSUM") as tps_pool, \
                 tc.tile_pool(name="hps", bufs=1, space="PSUM") as hps_pool, \
                 tc.tile_pool(name="rps", bufs=1, space="PSUM") as rps_pool, \
                 tc.tile_pool(name="yps", bufs=1, space="PSUM") as yps_pool, \
                 tc.tile_pool(name="kps", bufs=3, space="PSUM") as kps_pool, \
                 tc.tile_pool(name="work", bufs=2) as wpool, \
                 tc.tile_pool(name="bwork", bufs=4) as bpool, \
                 tc.tile_pool(name="kout", bufs=3) as kpool:
                for g in range(NG):
                    t0 = TB * g
                    # transpose 4 tiles x 10 channels -> [40, 128]
                    t_ps = tps_pool.tile([NCH * TB, 128], F32)
                    nc.tensor.transpose(
                        t_ps[:], chan[:, NCH * t0:NCH * t0 + NCH * TB], id_sb[:])
                    t_sb = wpool.tile([NCH * TB, 128], F32, tag="t_sb")
                    nc.scalar.copy(t_sb[:], t_ps[:])

                    # radial MLP hidden layer for the whole group
                    h_ps = hps_pool.tile([128, TB * 128], F32)
                    for dt in range(TB):
                        nc.tensor.matmul(
                            h_ps[:, dt * 128:(dt + 1) * 128],
                            w1e_sb[:, dt * 128:(dt + 1) * 128],
                            t_sb[:], start=True, stop=True)
                    h_sb = wpool.tile([128, TB * 128], F32, tag="h_sb")
                    nc.scalar.activation(h_sb[:], h_ps[:], ACTF.Relu, bias=b1_sb[:])

                    r_ps = rps_pool.tile([54, TB * 128], F32)
                    nc.tensor.matmul(r_ps[:], w2e_sb[:], h_sb[:],
                                     start=True, stop=True)
                    y_ps = yps_pool.tile([54, TB * 128], F32)
                    for dt in range(TB):
                        nc.tensor.matmul(
                            y_ps[:, dt * 128:(dt + 1) * 128],
                            ey4_sb[:, dt * 54:(dt + 1) * 54],
                            t_sb[:], start=True, stop=True)

                    # B = (R + b2) * Y', split hi/lo for the fp32r contraction
                    b1g = wpool.tile([54, TB * 128], F32, tag="b1g")
                    nc.vector.tensor_scalar(b1g[:], r_ps[:], b2_sb[:],
                                            None, ALU.add)

                    k_sb = kpool.tile([128, TB * IJ], F32, tag="k_sb")
                    for dt in range(TB):
                        bf = bpool.tile([54, 128], F32, tag="bf")
                        nc.vector.tensor_tensor(
                            bf[:], b1g[:, dt * 128:(dt + 1) * 128],
                            y_ps[:, dt * 128:(dt + 1) * 128], ALU.mult)
                        slot = (g * TB + dt) % 6
                        bstk = bstk_all[:, slot * 128:(slot + 1) * 128]
                        nc.vector.tensor_copy(bstk[0:54, :], bf[:])
                        nc.vector.tensor_tensor(
                            bstk[64:118, :], bf[:],
                            bstk[0:54, :].bitcast(F32), ALU.subtract)

                        for half in range(2):
                            k_ps = kps_pool.tile([128, 512], F32, tag="k_ps")
                            nc.tensor.matmul(
                                k_ps[:], bstk[:],
                                mstk[:, half * 512:(half + 1) * 512],
                                start=True, stop=False)
                            nc.tensor.matmul(
                                k_ps[:], bstk[0:54, :],
                                ml_sb[:, half * 512:(half + 1) * 512],
                                start=False, stop=True)
                            dest = k_sb[:, dt * IJ + half * 512:
                                        dt * IJ + (half + 1) * 512]
                            if (dt * 2 + half) % 4 == 3:
                                nc.vector.tensor_copy(dest, k_ps[:])
                            else:
                                nc.scalar.copy(dest, k_ps[:])

                    # store: group covers z rows [512g, 512g+512)
                    z0 = 512 * g
                    if z0 + 512 <= ZC:
                        for hfg in range(2):
                            og = out[z0 + hfg * 256:z0 + hfg * 256 + 256, :].rearrange(
                                "(dt pg) ij -> pg dt ij", dt=2)
                            nc.sync.dma_start(
                                og, k_sb[:, hfg * 2 * IJ:(hfg + 1) * 2 * IJ].rearrange(
                                    "pg (dt ij) -> pg dt ij", dt=2))
                    else:
                        # last group: tiles beyond ZC are padding
                        for dt in range(TB):
                            zt = z0 + dt * 128
                            if zt >= ZC:
                                break
                            rows = min(128, ZC - zt)
                            nc.sync.dma_start(
                                out[zt:zt + rows, :],
                                k_sb[0:rows, dt * IJ:(dt + 1) * IJ])
    nc.compile()
    return nc


def _get_program():
    if "nc" not in _CACHE:
        _CACHE["nc"] = _build_program()
    return _CACHE["nc"]


def _host_prep(r, W1, b1, W2, b2, cg, ylm_mix, rf_mix, norm_coef):
    r = np.asarray(r, dtype=np.float32)
    W1 = np.asarray(W1, dtype=np.float32)
    b1 = np.asarray(b1, dtype=np.float32)
    W2 = np.asarray(W2, dtype=np.float32)
    b2 = np.asarray(b2, dtype=np.float32)
    cg = np.asarray(cg, dtype=np.float32)
    ylm_mix = np.asarray(ylm_mix, dtype=np.float32)
    rf_mix = np.asarray(rf_mix, dtype=np.float32)
    norm_coef = np.asarray(norm_coef, dtype=np.float32)

    w1e4 = np.zeros((NCH * TB, TB * 128), dtype=np.float32)
    ey4 = np.zeros((NCH * TB, TB * 54), dtype=np.float32)
    for dt in range(TB):
        w1e4[NCH * dt, dt * 128:(dt + 1) * 128] = W1[0]
        for l in range(9):
            for p in range(NPATH):
                ey4[NCH * dt + 1 + l, dt * 54 + p * 9 + l] = 1.0

    ylm_s = (ylm_mix.astype(np.float64) * YLM_SCALE[None, :]).astype(np.float32)
    shared = {
        "w1e4": w1e4,
        "ey4": ey4,
        "w2e": np.ascontiguousarray(np.repeat(W2, 9, axis=1)),
        "b1c": np.ascontiguousarray(b1.reshape(H, 1)),
        "b2r": np.ascontiguousarray(np.repeat(b2, 9).reshape(54, 1)),
        "cgd": np.ascontiguousarray(cg.reshape(KDIM, IJ)),
        "rft": np.ascontiguousarray(
            rf_mix.reshape(NKT, 128, NPATH).transpose(1, 0, 2).reshape(128, NKT * NPATH)),
        "ylt": np.ascontiguousarray(
            ylm_s.reshape(NKT, 128, 9).transpose(1, 0, 2).reshape(128, NKT * 9)),
        "ncv": np.ascontiguousarray(norm_coef[:, :, 0].reshape(1, IJ)),
        "identd": np.eye(128, dtype=np.float32),
    }

    in_maps = []
    for c in range(NCORES):
        rs = r[c * ZC:(c + 1) * ZC]
        rp = np.empty((ZC_PAD, 3), dtype=np.float32)
        rp[:ZC] = rs
        rp[ZC:] = np.array([1.0, 0.0, 0.0], dtype=np.float32)
        rpl = rp.reshape(T, 128, 3).transpose(1, 2, 0).reshape(128, 3 * T)
        m = dict(shared)
        m["rpl"] = np.ascontiguousarray(rpl)
        in_maps.append(m)
    return in_maps


def _run_device(in_maps, trace=False, **kw):
    nc = _get_program()
    return run_bass_kernel_spmd(nc, in_maps, core_ids=list(range(NCORES)),
                                trace=trace, **kw)


def kernel(r, W1, b1, W2, b2, cg, ylm_mix, rf_mix, norm_coef):
    r = np.asarray(r, dtype=np.float32)
    norm_coef_f = np.asarray(norm_coef, dtype=np.float32)
    in_maps = _host_prep(r, W1, b1, W2, b2, cg, ylm_mix, rf_mix, norm_coef_f)
    res = _run_device(in_maps)
    out = np.concatenate([res.results[c]["out"] for c in range(NCORES)], axis=0)

    # points with exactly zero radius use norm_coef[..., 1] instead of [..., 0]
    x, y, z = r[:, 0], r[:, 1], r[:, 2]
    r2 = (x * x + y * y) + z * z
    zero = r2 == np.float32(0.0)
    if np.any(zero):
        scale = (norm_coef_f[:, :, 1].astype(np.float64)
                 / norm_coef_f[:, :, 0].astype(np.float64)).reshape(1, IJ)
        out[zero] = (out[zero].astype(np.float64) * scale).astype(np.float32)

    return out.reshape(Z, DO, DI)



# revision 16
# speedup vs baseline: 1.6767x; 1.6767x over previous
"""Trainium2 Bass kernel for the gnn_message_passing problem.

Math refactor: the reference computes
    kernel[z,i,j] = einsum('zk,kij->zij', Rk*Yk, cg) * nc0[i,j]
with Rk = R @ rf_mix.T (rank 6) and Yk = Y.T @ ylm_mix.T (rank 9).
Fold the K=1024 contraction into a host-precomputed constant
    M[p*9+l, ij] = sum_k rf[k,p]*ylm_s[k,l]*cg[k,ij] * nc0[ij]
so each point only needs B[z, pl] = (R[z,p]+b2[p]) * Y'[z,l] contracted
against M - a k=64 fp32r matmul per 128-point tile.  The output is
written to DRAM in bf16 (halving the dominant HBM traffic) and widened
to fp32 on the host; bf16 keeps the max relative error ~2e-3, well
inside the 2e-2 gate.

Device pipeline per 512-point group (4 tiles of 128):
  PE   : per-pair channel transposes, radial-MLP hidden matmul,
         radial matmul, 8 k=64 output matmuls (all fp32r, N=512)
  ACT  : relu (bias b1 fused), 2 PSUM->SBUF bf16 tile copies
  DVE  : fused B build ((R+b2)*Y via one scalar_tensor_tensor),
         2 PSUM->SBUF bf16 tile copies
  DMA  : tps PSUM->SBUF bounce (row-priced, cheap), 1 MB output store

The channel planes hold the 9 SH ratios pre-replicated across the 6
radial paths (plus plain ratios and radii), so the per-group transpose
directly yields the [64, points] operand layout and no per-group
replication work is needed.

Distribution: data-parallel over z across 8 NeuronCores; constants
replicated. Full inputs in, full output out.
"""

import numpy as np

import concourse.bass as bass
import concourse.tile as tile
from concourse import bacc, mybir
from concourse.bass_utils import run_bass_kernel_spmd

F32 = mybir.dt.float32
F32R = mybir.dt.float32r
BF16 = mybir.dt.bfloat16
ALU = mybir.AluOpType
ACTF = mybir.ActivationFunctionType

# The transposed-channel path runs in bf16 (2^-9 relative rounding; the
# rel-err budget is 2e-2): PE transposes at 1 cycle/row, the pair-transpose
# PSUM tile is a single bank, and the per-group tps bounce uses the DVE 2x
# packed mode.  The walrus BIR verifier requires every FP32r matmult operand
# to be PRODUCED by an engine op with f32r output dtype (a DMA of f32 bits
# does not count), so M and W2 are staged through a one-time engine copy.

# Problem shape (hardcoded per contract)
Z, KDIM, DO, DI, NPATH, H = 100000, 1024, 32, 32, 6, 128
IJ = DO * DI                      # 1024
NCORES = 8
ZC = Z // NCORES                  # 12500 points per core
T = 100                           # point tiles of 128 -> ZC padded to 12800
ZC_PAD = 128 * T
TB = 4                            # tiles per group
NG = T // TB                      # 25 groups
W = 64                            # transposed channel rows: 54 rep + 9 plain + radii
NQ = 64                           # B-stack rows

# Real spherical harmonic constants (l=0,1,2), folded into M host-side
C0 = 0.28209479177387814
C1 = 0.4886025119029199
C2A = 1.0925484305920792
C2B = 0.31539156525252005
C2C = 0.5462742152960396
YLM_SCALE = np.array([C0, C1, C1, C1, C2A, C2A, C2B, C2A, C2C], dtype=np.float64)

_CACHE = {}


def _build_program():
    nc = bacc.Bacc("TRN2", target_bir_lowering=False, debug=False,
                   num_devices=NCORES)

    # ---- per-core DRAM I/O ----
    rpl = nc.dram_tensor("rpl", [128, 3 * T], F32, kind="ExternalInput").ap()
    m64d = nc.dram_tensor("m64d", [NQ, IJ], F32, kind="ExternalInput").ap()
    w1e2d = nc.dram_tensor("w1e2d", [W, 128], BF16, kind="ExternalInput").ap()
    w2e64d = nc.dram_tensor("w2e64d", [H, NQ], F32, kind="ExternalInput").ap()
    b1d = nc.dram_tensor("b1d", [H, 1], F32, kind="ExternalInput").ap()
    addvd = nc.dram_tensor("addvd", [NQ, 1], F32, kind="ExternalInput").ap()
    identd = nc.dram_tensor("identd", [128, 128], BF16, kind="ExternalInput").ap()
    out = nc.dram_tensor("out", [ZC, IJ], BF16, kind="ExternalOutput").ap()

    with tile.TileContext(nc) as tc:
        with tc.tile_pool(name="const", bufs=1) as cpool:
            # ---- resident constants ----
            m_stage = cpool.tile([NQ, IJ], F32)
            nc.sync.dma_start(m_stage[:], m64d[:])
            m_sb = cpool.tile([NQ, IJ], F32R)
            nc.vector.tensor_copy(m_sb[:], m_stage[:])
            w1_sb = cpool.tile([W, 128], BF16)
            nc.sync.dma_start(w1_sb[:], w1e2d[:])
            w2_stage = cpool.tile([H, NQ], F32)
            nc.sync.dma_start(w2_stage[:], w2e64d[:])
            w2_sb = cpool.tile([H, NQ], F32R)
            nc.vector.tensor_copy(w2_sb[:], w2_stage[:])
            b1_sb = cpool.tile([H, 1], F32)
            nc.sync.dma_start(b1_sb[:], b1d[:])
            addv_sb = cpool.tile([NQ, 1], F32)
            nc.sync.dma_start(addv_sb[:], addvd[:])
            id_sb = cpool.tile([128, 128], BF16)
            nc.sync.dma_start(id_sb[:], identd[:])
            rpl_sb = cpool.tile([128, 3 * T], F32)
            nc.sync.dma_start(rpl_sb[:], rpl[:])

            x = rpl_sb[:, 0:T]
            y = rpl_sb[:, T:2 * T]
            z = rpl_sb[:, 2 * T:3 * T]

            # =========================================================
            # Prologue: channel planes.  chan col layout per tile t:
            #   t*64 + p*9 + l : ratio_l (replicated per path p)
            #   t*64 + 54 + l  : ratio_l (plain; l=0 is the ones channel)
            #   t*64 + 63      : radii
            # =========================================================
            chan = cpool.tile([128, W * T], BF16)
            aux = cpool.tile([128, 8 * T], F32)

            def ax(i):
                return aux[:, i * T:(i + 1) * T]

            xx, yy, zz, r2, safe, inv2, invr, tmp = (ax(i) for i in range(8))
            chv = chan[:].rearrange("p (t c) -> p t c", c=W)

            def pl(j):   # plain channel column j (one col per tile)
                return chv[:, :, j]

            nc.vector.tensor_tensor(xx, x, x, ALU.mult)
            nc.vector.tensor_tensor(yy, y, y, ALU.mult)
            nc.vector.tensor_tensor(zz, z, z, ALU.mult)
            nc.vector.tensor_tensor(tmp, xx, yy, ALU.add)
            nc.vector.tensor_tensor(r2, tmp, zz, ALU.add)
            # guard r2 == 0 exactly like the reference's safe_r2
            nc.vector.tensor_scalar(safe, r2, 0.0, None, ALU.is_equal)
            nc.vector.tensor_tensor(safe, r2, safe, ALU.add)
            nc.vector.reciprocal(inv2, safe)              # 1/safe_r2
            nc.scalar.sqrt(invr, inv2)                    # 1/safe_r (~7e-6 rel)

            nc.vector.memset(pl(54), 1.0)                 # ones (l=0)
            nc.vector.tensor_tensor(pl(55), y, invr, ALU.mult)
            nc.vector.tensor_tensor(pl(56), z, invr, ALU.mult)
            nc.vector.tensor_tensor(pl(57), x, invr, ALU.mult)
            nc.vector.tensor_tensor(tmp, x, y, ALU.mult)
            nc.vector.tensor_tensor(pl(58), tmp, inv2, ALU.mult)
            nc.vector.tensor_tensor(tmp, y, z, ALU.mult)
            nc.vector.tensor_tensor(pl(59), tmp, inv2, ALU.mult)
            nc.vector.scalar_tensor_tensor(tmp, zz, 3.0, r2, ALU.mult,
                                           ALU.subtract)
            nc.vector.tensor_tensor(pl(60), tmp, inv2, ALU.mult)
            nc.vector.tensor_tensor(tmp, x, z, ALU.mult)
            nc.vector.tensor_tensor(pl(61), tmp, inv2, ALU.mult)
            nc.vector.tensor_tensor(tmp, xx, yy, ALU.subtract)
            nc.vector.tensor_tensor(pl(62), tmp, inv2, ALU.mult)
            nc.vector.tensor_tensor(pl(63), r2, invr, ALU.mult)   # radii

            # replicate the 9 plain ratios into the 6 path blocks
            plain9 = chv[:, :, 54:63]
            for p in range(NPATH):
                nc.vector.tensor_copy(chv[:, :, p * 9:p * 9 + 9], plain9)

            # =========================================================
            # Main loop: 25 groups of 4 tiles; transposes batched in
            # pairs of groups (8 tiles -> [64, 1024] PSUM -> DMA bounce)
            # =========================================================
            NPAIR = (NG + 1) // 2     # 13 (last pair is half-size)

            with tc.tile_pool(name="tps", bufs=2, space="PSUM") as tpool, \
                 tc.tile_pool(name="hps", bufs=1, space="PSUM") as hpool, \
                 tc.tile_pool(name="rps", bufs=1, space="PSUM") as rpool, \
                 tc.tile_pool(name="kps", bufs=2, space="PSUM") as kpool, \
                 tc.tile_pool(name="tsb", bufs=2) as tspool, \
                 tc.tile_pool(name="work", bufs=2) as wpool, \
                 tc.tile_pool(name="kout", bufs=3) as opool:

                tp_ps = {}
                tps_sb = {}

                def emit_pair(pr):
                    # transpose 8 (or 4) tiles' channels into PSUM
                    t0 = pr * 2 * TB
                    ntile = min(2 * TB, T - t0)
                    tp = tpool.tile([W, 2 * TB * 128], BF16, tag="tp")
                    for i in range(ntile):
                        nc.tensor.transpose(
                            tp[:, i * 128:(i + 1) * 128],
                            chan[:, (t0 + i) * W:(t0 + i + 1) * W],
                            id_sb[:])
                    tp_ps[pr] = tp
                    tps_sb[pr] = tspool.tile([W, 2 * TB * 128], BF16,
                                             tag="tsb", name="tps_sb")

                def emit_tpscopy(g):
                    # bounce group g's tps half PSUM->SBUF (emitted one
                    # iteration ahead so h never waits on it in-iteration)
                    pr, hf = g // 2, g % 2
                    tsl = tps_sb[pr][:, hf * 512:hf * 512 + 512]
                    nc.vector.tensor_copy(
                        tsl, tp_ps[pr][:, hf * 512:hf * 512 + 512])
                    return tsl

                def emit_front(g, tsl):
                    # radial MLP hidden layer + relu
                    h_ps = hpool.tile([128, 512], F32, tag="hps")
                    nc.tensor.matmul(h_ps[:], w1_sb[:], tsl,
                                     start=True, stop=True)
                    h_sb = wpool.tile([128, 512], F32R, tag="hsb")
                    nc.scalar.activation(h_sb[:], h_ps[:], ACTF.Relu,
                                         bias=b1_sb[:])
                    return h_sb

                def emit_r(g, h_sb):
                    r_ps = rpool.tile([NQ, 512], F32, tag="rps")
                    nc.tensor.matmul(r_ps[:], w2_sb[:], h_sb[:],
                                     start=True, stop=True)
                    return r_ps

                def emit_bmult(g, tsl, r_ps):
                    # fused B build: bstk = (R + addv) * Y  (one DVE op)
                    bstk = wpool.tile([NQ, 512], F32R, tag="bstk")
                    nc.vector.scalar_tensor_tensor(
                        bstk[:], r_ps[:], addv_sb[:], tsl,
                        ALU.add, ALU.mult)
                    return bstk

                def emit_main(g, bstk, dts):
                    # k=64 output matmuls for tiles `dts` of group g
                    tiles = []
                    for dt in dts:
                        k_ps = kpool.tile([128, IJ], F32, tag="kps")
                        for half in range(2):
                            nc.tensor.matmul(
                                k_ps[:, half * 512:(half + 1) * 512],
                                bstk[:, dt * 128:(dt + 1) * 128],
                                m_sb[:, half * 512:(half + 1) * 512],
                                start=True, stop=True)
                        tiles.append(k_ps)
                    return tiles

                def emit_copy(k_sb, dt, k_ps):
                    # PSUM f32 -> SBUF bf16; split tuned for DVE/ACT
                    # balance.  dt0 is split across both engines so its
                    # k-PSUM slot frees early (dt2 reuses it).
                    dst = k_sb[:, dt * IJ:(dt + 1) * IJ]
                    if dt == 0:
                        nc.vector.tensor_copy(dst[:, 0:512], k_ps[:, 0:512])
                        nc.scalar.copy(dst[:, 512:IJ], k_ps[:, 512:IJ])
                    elif dt == 2:
                        nc.vector.tensor_copy(dst[:], k_ps[:])
                    else:
                        nc.scalar.copy(dst[:], k_ps[:])

                def emit_store(g, k_sb):
                    z0 = 512 * g
                    if z0 + 512 <= ZC:
                        og = out[z0:z0 + 512, :].rearrange(
                            "(dt p) ij -> p dt ij", dt=TB)
                        nc.sync.dma_start(
                            og, k_sb[:].rearrange("p (dt ij) -> p dt ij",
                                                  dt=TB))
                    else:
                        for dt in range(TB):
                            zt = z0 + dt * 128
                            if zt >= ZC:
                                break
                            rows = min(128, ZC - zt)
                            nc.sync.dma_start(
                                out[zt:zt + rows, :],
                                k_sb[0:rows, dt * IJ:(dt + 1) * IJ])

                # Software-pipelined emission.  PE order per iteration:
                # [h_g] [main g-1: dt0,dt1] [R_g] [main g-1: dt2,dt3]
                # [pair transposes] — R_g slots into the k-PSUM reuse
                # window so the PE never idles on output copies, and the
                # tps bounce for group g+1 is issued at the tail of this
                # iteration's DVE work so h_{g+1} starts immediately.
                emit_pair(0)
                tsl = emit_tpscopy(0)
                prev = None      # (bstk, tsl) of group g-1
                for g in range(NG):
                    h_sb = emit_front(g, tsl)
                    if prev is not None:
                        pg, pb = prev
                        pk_sb = opool.tile([128, TB * IJ], BF16, tag="ksb")
                        t01 = emit_main(pg, pb, (0, 1))
                        emit_copy(pk_sb, 0, t01[0])
                        r_ps = emit_r(g, h_sb)
                        emit_copy(pk_sb, 1, t01[1])
                        t23 = emit_main(pg, pb, (2, 3))
                        emit_copy(pk_sb, 2, t23[0])
                        emit_copy(pk_sb, 3, t23[1])
                        emit_store(pg, pk_sb)
                    else:
                        r_ps = emit_r(g, h_sb)
                    bstk = emit_bmult(g, tsl, r_ps)
                    if g % 2 == 1 and g // 2 + 1 < NPAIR:
                        emit_pair(g // 2 + 1)
                    if g + 1 < NG:
                        tsl = emit_tpscopy(g + 1)
                    prev = (g, bstk)
                pg, pb = prev
                pk_sb = opool.tile([128, TB * IJ], BF16, tag="ksb")
                for dt, kt in zip(range(TB), emit_main(pg, pb, (0, 1, 2, 3))):
                    emit_copy(pk_sb, dt, kt)
                emit_store(pg, pk_sb)

    nc.compile()
    return nc


def _get_program():
    if "nc" not in _CACHE:
        _CACHE["nc"] = _build_program()
    return _CACHE["nc"]


def _host_prep(r, W1, b1, W2, b2, cg, ylm_mix, rf_mix, norm_coef):
    r = np.asarray(r, dtype=np.float32)
    W1 = np.asarray(W1, dtype=np.float32)
    b1 = np.asarray(b1, dtype=np.float32)
    W2 = np.asarray(W2, dtype=np.float32)
    b2 = np.asarray(b2, dtype=np.float32)
    cg = np.asarray(cg, dtype=np.float32)
    ylm_mix = np.asarray(ylm_mix, dtype=np.float32)
    rf_mix = np.asarray(rf_mix, dtype=np.float32)
    norm_coef = np.asarray(norm_coef, dtype=np.float32)

    # constant folding: M64[p*9+l, ij] = sum_k rf[k,p] ylm_s[k,l] cg[k,ij] * nc0
    ylm_s = ylm_mix.astype(np.float64) * YLM_SCALE[None, :]
    wkp = rf_mix.astype(np.float64)[:, :, None] * ylm_s[:, None, :]  # [K,p,l]
    mcore = np.einsum("kq,kj->qj", wkp.reshape(KDIM, 54),
                      cg.astype(np.float64).reshape(KDIM, IJ))
    nc0 = norm_coef.astype(np.float64)[:, :, 0].reshape(1, IJ)
    m64 = np.zeros((NQ, IJ), dtype=np.float32)
    m64[0:54] = (mcore * nc0).astype(np.float32)

    import ml_dtypes
    w1e2 = np.zeros((W, 128), dtype=ml_dtypes.bfloat16)
    w1e2[63, :] = W1[0].astype(ml_dtypes.bfloat16)

    w2e64 = np.zeros((H, NQ), dtype=np.float32)
    w2e64[:, 0:54] = np.repeat(W2, 9, axis=1)

    addv = np.zeros((NQ, 1), dtype=np.float32)
    addv[0:54, 0] = np.repeat(b2, 9)
    addv[54:63, 0] = 1.0

    shared = {
        "m64d": m64,
        "w1e2d": w1e2,
        "w2e64d": w2e64,
        "b1d": np.ascontiguousarray(b1.reshape(H, 1)),
        "addvd": addv,
        "identd": np.eye(128, dtype=ml_dtypes.bfloat16),
    }

    in_maps = []
    for c in range(NCORES):
        rs = r[c * ZC:(c + 1) * ZC]
        rp = np.empty((ZC_PAD, 3), dtype=np.float32)
        rp[:ZC] = rs
        rp[ZC:] = np.array([1.0, 0.0, 0.0], dtype=np.float32)
        rpl = rp.reshape(T, 128, 3).transpose(1, 2, 0).reshape(128, 3 * T)
        m = dict(shared)
        m["rpl"] = np.ascontiguousarray(rpl)
        in_maps.append(m)
    return in_maps


def _run_device(in_maps, trace=False, **kw):
    nc = _get_program()
    return run_bass_kernel_spmd(nc, in_maps, core_ids=list(range(NCORES)),
                                trace=trace, **kw)


def kernel(r, W1, b1, W2, b2, cg, ylm_mix, rf_mix, norm_coef):
    r = np.asarray(r, dtype=np.float32)
    norm_coef_f = np.asarray(norm_coef, dtype=np.float32)
    in_maps = _host_prep(r, W1, b1, W2, b2, cg, ylm_mix, rf_mix, norm_coef_f)
    res = _run_device(in_maps)
    out = np.concatenate(
        [np.asarray(res.results[c]["out"]).astype(np.float32)
         for c in range(NCORES)], axis=0)

    # points with exactly zero radius use norm_coef[..., 1] instead of [..., 0]
    x, y, z = r[:, 0], r[:, 1], r[:, 2]
    r2 = (x * x + y * y) + z * z
    zero = r2 == np.float32(0.0)
    if np.any(zero):
        scale = (norm_coef_f[:, :, 1].astype(np.float64)
                 / norm_coef_f[:, :, 0].astype(np.float64)).reshape(1, IJ)
        out[zero] = (out[zero].astype(np.float64) * scale).astype(np.float32)

    return out.reshape(Z, DO, DI)


# revision 34
# speedup vs baseline: 1.7650x; 1.0527x over previous
"""Trainium2 Bass kernel for the gnn_message_passing problem.

Math refactor: the reference computes
    kernel[z,i,j] = einsum('zk,kij->zij', Rk*Yk, cg) * nc0[i,j]
with Rk = R @ rf_mix.T (rank 6) and Yk = Y.T @ ylm_mix.T (rank 9).
Fold the K=1024 contraction into a host-precomputed constant
    M[p*9+l, ij] = sum_k rf[k,p]*ylm_s[k,l]*cg[k,ij] * nc0[ij]
so each point only needs B[z, pl] = (R[z,p]+b2[p]) * Y'[z,l] contracted
against M - a k=64 fp32r matmul per 128-point tile.  The output is
written to DRAM in bf16 (halving the dominant HBM traffic) and widened
to fp32 on the host; bf16 keeps the max relative error ~2e-3, well
inside the 2e-2 gate.

Device pipeline per 512-point group (4 tiles of 128):
  PE   : per-pair channel transposes, radial-MLP hidden matmul,
         radial matmul, 8 k=64 output matmuls (all fp32r, N=512)
  ACT  : relu (bias b1 fused), 2 PSUM->SBUF bf16 tile copies
  DVE  : fused B build ((R+b2)*Y via one scalar_tensor_tensor),
         2 PSUM->SBUF bf16 tile copies
  DMA  : tps PSUM->SBUF bounce (row-priced, cheap), 1 MB output store

The channel planes hold the 9 SH ratios pre-replicated across the 6
radial paths (plus plain ratios and radii), so the per-group transpose
directly yields the [64, points] operand layout and no per-group
replication work is needed.

Distribution: data-parallel over z across 8 NeuronCores; constants
replicated. Full inputs in, full output out.
"""

import numpy as np

import concourse.bass as bass
import concourse.tile as tile
from concourse import bacc, mybir
from concourse.bass_utils import run_bass_kernel_spmd

F32 = mybir.dt.float32
F32R = mybir.dt.float32r
BF16 = mybir.dt.bfloat16
ALU = mybir.AluOpType
ACTF = mybir.ActivationFunctionType

# The transposed-channel path runs in bf16 (2^-9 relative rounding; the
# rel-err budget is 2e-2): PE transposes at 1 cycle/row, the pair-transpose
# PSUM tile is a single bank, and the per-group tps bounce uses the DVE 2x
# packed mode.  The walrus BIR verifier requires every FP32r matmult operand
# to be PRODUCED by an engine op with f32r output dtype (a DMA of f32 bits
# does not count), so M and W2 are staged through a one-time engine copy.

# Problem shape (hardcoded per contract)
Z, KDIM, DO, DI, NPATH, H = 100000, 1024, 32, 32, 6, 128
IJ = DO * DI                      # 1024
NCORES = 8
ZC = Z // NCORES                  # 12500 points per core
T = 100                           # point tiles of 128 -> ZC padded to 12800
ZC_PAD = 128 * T
TB = 4                            # tiles per group
NG = T // TB                      # 25 groups
W = 64                            # transposed channel rows: 54 rep + 9 plain + radii
NQ = 64                           # B-stack rows

# Real spherical harmonic constants (l=0,1,2), folded into M host-side
C0 = 0.28209479177387814
C1 = 0.4886025119029199
C2A = 1.0925484305920792
C2B = 0.31539156525252005
C2C = 0.5462742152960396
YLM_SCALE = np.array([C0, C1, C1, C1, C2A, C2A, C2B, C2A, C2C], dtype=np.float64)

_CACHE = {}


def _build_program():
    nc = bacc.Bacc("TRN2", target_bir_lowering=False, debug=False,
                   num_devices=NCORES)

    # ---- per-core DRAM I/O ----
    rpl = nc.dram_tensor("rpl", [128, 3 * T], F32, kind="ExternalInput").ap()
    m64d = nc.dram_tensor("m64d", [NQ, IJ], F32, kind="ExternalInput").ap()
    w1e2d = nc.dram_tensor("w1e2d", [W, 128], BF16, kind="ExternalInput").ap()
    w2e64d = nc.dram_tensor("w2e64d", [H, NQ], F32, kind="ExternalInput").ap()
    b1d = nc.dram_tensor("b1d", [H, 1], F32, kind="ExternalInput").ap()
    addvd = nc.dram_tensor("addvd", [NQ, 1], F32, kind="ExternalInput").ap()
    identd = nc.dram_tensor("identd", [128, 128], BF16, kind="ExternalInput").ap()
    out = nc.dram_tensor("out", [ZC, IJ], BF16, kind="ExternalOutput").ap()

    with tile.TileContext(nc) as tc:
        with tc.tile_pool(name="const", bufs=1) as cpool:
            # ---- resident constants (rpl first: everything downstream
            # of the monomial prologue waits on it) ----
            rpl_sb = cpool.tile([128, 3 * T], F32)
            nc.sync.dma_start(rpl_sb[:], rpl[:])
            m_stage = cpool.tile([NQ, IJ], F32)
            nc.sync.dma_start(m_stage[:], m64d[:])
            m_sb = cpool.tile([NQ, IJ], F32R)
            w1_sb = cpool.tile([W, 128], BF16)
            nc.sync.dma_start(w1_sb[:], w1e2d[:])
            w2_stage = cpool.tile([H, NQ], F32)
            nc.sync.dma_start(w2_stage[:], w2e64d[:])
            w2_sb = cpool.tile([H, NQ], F32R)
            b1_sb = cpool.tile([H, 1], F32)
            nc.sync.dma_start(b1_sb[:], b1d[:])
            addv_sb = cpool.tile([NQ, 1], F32)
            nc.sync.dma_start(addv_sb[:], addvd[:])
            id_sb = cpool.tile([128, 128], BF16)
            nc.sync.dma_start(id_sb[:], identd[:])

            x = rpl_sb[:, 0:T]
            y = rpl_sb[:, T:2 * T]
            z = rpl_sb[:, 2 * T:3 * T]

            # =========================================================
            # Prologue: channel planes.  chan col layout per tile t:
            #   t*64 + p*9 + l : ratio_l (replicated per path p)
            #   t*64 + 54 + l  : ratio_l (plain; l=0 is the ones channel)
            #   t*64 + 63      : radii
            # =========================================================
            chan = cpool.tile([128, W * T], BF16)
            aux = cpool.tile([128, 12 * T], F32)

            def ax(i):
                return aux[:, i * T:(i + 1) * T]

            (xx, yy, zz, r2, safe, inv2, invr, tmp,
             tmp2, tmp3, tmp4, tmp5) = (ax(i) for i in range(12))
            chv = chan[:].rearrange("p (t c) -> p t c", c=W)

            def pl(j):   # plain channel column j (one col per tile)
                return chv[:, :, j]

            nc.vector.tensor_tensor(xx, x, x, ALU.mult)
            nc.vector.tensor_tensor(yy, y, y, ALU.mult)
            nc.vector.tensor_tensor(zz, z, z, ALU.mult)
            nc.vector.tensor_tensor(tmp, xx, yy, ALU.add)
            nc.vector.tensor_tensor(r2, tmp, zz, ALU.add)
            # guard r2 == 0 exactly like the reference's safe_r2
            nc.vector.tensor_scalar(safe, r2, 0.0, None, ALU.is_equal)
            nc.vector.tensor_tensor(safe, r2, safe, ALU.add)
            nc.vector.reciprocal(inv2, safe)              # 1/safe_r2
            nc.scalar.sqrt(invr, inv2)                    # 1/safe_r (~7e-6 rel)

            # products that only need x/y/z go to Pool in parallel with
            # the DVE chain (GPSIMD has no PSUM port but SBUF ops are fine)
            nc.vector.memset(pl(54), 1.0)                 # ones (l=0)
            nc.vector.tensor_tensor(pl(55), y, invr, ALU.mult)
            nc.vector.tensor_tensor(pl(56), z, invr, ALU.mult)
            nc.vector.tensor_tensor(pl(57), x, invr, ALU.mult)
            nc.vector.tensor_tensor(tmp2, x, y, ALU.mult)
            nc.vector.tensor_tensor(pl(58), tmp2, inv2, ALU.mult)
            nc.vector.tensor_tensor(tmp3, y, z, ALU.mult)
            nc.vector.tensor_tensor(pl(59), tmp3, inv2, ALU.mult)
            nc.vector.scalar_tensor_tensor(tmp, zz, 3.0, r2, ALU.mult,
                                           ALU.subtract)
            nc.vector.tensor_tensor(pl(60), tmp, inv2, ALU.mult)
            nc.vector.tensor_tensor(tmp4, x, z, ALU.mult)
            nc.vector.tensor_tensor(pl(61), tmp4, inv2, ALU.mult)
            nc.vector.tensor_tensor(tmp5, xx, yy, ALU.subtract)
            nc.vector.tensor_tensor(pl(62), tmp5, inv2, ALU.mult)
            nc.vector.tensor_tensor(pl(63), r2, invr, ALU.mult)   # radii

            # replicate the 9 plain ratios into the 6 path blocks; the
            # first pair's 8 tiles go first so its transposes start while
            # the remaining 92 tiles replicate, and the one-time f32r
            # weight staging copies slot in between (off the critical path)
            chv8 = chan[:, 0:8 * W].rearrange("p (t c) -> p t c", c=W)
            chv92 = chan[:, 8 * W:].rearrange("p (t c) -> p t c", c=W)
            for p in range(NPATH):
                nc.vector.tensor_copy(chv8[:, :, p * 9:p * 9 + 9],
                                      chv8[:, :, 54:63])
            nc.vector.tensor_copy(w2_sb[:], w2_stage[:])
            nc.vector.tensor_copy(m_sb[:], m_stage[:])
            for p in range(NPATH):
                nc.vector.tensor_copy(chv92[:, :, p * 9:p * 9 + 9],
                                      chv92[:, :, 54:63])

            # =========================================================
            # Main loop: 25 groups of 4 tiles; transposes batched in
            # pairs of groups (8 tiles -> [64, 1024] PSUM -> DMA bounce)
            # =========================================================
            NPAIR = (NG + 1) // 2     # 13 (last pair is half-size)

            with tc.tile_pool(name="tps", bufs=1, space="PSUM") as tpool, \
                 tc.tile_pool(name="hps", bufs=1, space="PSUM") as hpool, \
                 tc.tile_pool(name="rps", bufs=2, space="PSUM") as rpool, \
                 tc.tile_pool(name="kps", bufs=2, space="PSUM") as kpool, \
                 tc.tile_pool(name="tsb", bufs=2) as tspool, \
                 tc.tile_pool(name="work", bufs=2) as wpool, \
                 tc.tile_pool(name="kout", bufs=3) as opool:

                tp_ps = {}
                tps_sb = {}

                def emit_pair(pr):
                    # transpose 8 (or 4) tiles' channels into PSUM
                    t0 = pr * 2 * TB
                    ntile = min(2 * TB, T - t0)
                    tp = tpool.tile([W, 2 * TB * 128], BF16, tag="tp")
                    for i in range(ntile):
                        nc.tensor.transpose(
                            tp[:, i * 128:(i + 1) * 128],
                            chan[:, (t0 + i) * W:(t0 + i + 1) * W],
                            id_sb[:])
                    tp_ps[pr] = tp
                    tps_sb[pr] = tspool.tile([W, 2 * TB * 128], BF16,
                                             tag="tsb", name="tps_sb")

                def emit_tpscopy(g):
                    # bounce group g's tps half PSUM->SBUF (DVE 2x packed)
                    pr, hf = g // 2, g % 2
                    tsl = tps_sb[pr][:, hf * 512:hf * 512 + 512]
                    nc.vector.tensor_copy(
                        tsl, tp_ps[pr][:, hf * 512:hf * 512 + 512])
                    return tsl

                def emit_front(g, tsl):
                    # radial MLP hidden layer + relu
                    h_ps = hpool.tile([128, 512], F32, tag="hps")
                    nc.tensor.matmul(h_ps[:], w1_sb[:], tsl,
                                     start=True, stop=True)
                    h_sb = wpool.tile([128, 512], F32R, tag="hsb")
                    nc.scalar.activation(h_sb[:], h_ps[:], ACTF.Relu,
                                         bias=b1_sb[:])
                    return h_sb

                def emit_r(g, h_sb):
                    r_ps = rpool.tile([NQ, 512], F32, tag="rps")
                    nc.tensor.matmul(r_ps[:], w2_sb[:], h_sb[:],
                                     start=True, stop=True)
                    return r_ps

                def emit_bmult(g, tsl, r_ps):
                    # fused B build: bstk = (R + addv) * Y  (one DVE op)
                    bstk = wpool.tile([NQ, 512], F32R, tag="bstk")
                    nc.vector.scalar_tensor_tensor(
                        bstk[:], r_ps[:], addv_sb[:], tsl,
                        ALU.add, ALU.mult)
                    return bstk

                def emit_main(g, bstk, dts):
                    # k=64 output matmuls for tiles `dts` of group g
                    tiles = []
                    for dt in dts:
                        k_ps = kpool.tile([128, IJ], F32, tag="kps")
                        for half in range(2):
                            nc.tensor.matmul(
                                k_ps[:, half * 512:(half + 1) * 512],
                                bstk[:, dt * 128:(dt + 1) * 128],
                                m_sb[:, half * 512:(half + 1) * 512],
                                start=True, stop=True)
                        tiles.append(k_ps)
                    return tiles

                def emit_copy(k_sb, dt, k_ps):
                    # PSUM f32 -> SBUF bf16; dt0 split across both engines
                    # so its k-PSUM slot frees early (dt2 reuses it)
                    dst = k_sb[:, dt * IJ:(dt + 1) * IJ]
                    if dt == 0:
                        nc.vector.tensor_copy(dst[:, 0:512], k_ps[:, 0:512])
                        nc.scalar.copy(dst[:, 512:IJ], k_ps[:, 512:IJ])
                    elif dt == 2:
                        nc.vector.tensor_copy(dst[:], k_ps[:])
                    else:
                        nc.scalar.copy(dst[:], k_ps[:])

                def emit_store(g, k_sb, hf):
                    # store half hf (2 tiles = 256 points) of group g; the
                    # first half is issued as soon as dt0/dt1 are copied so
                    # the DMA engines never sit idle waiting for dt3
                    z0 = 512 * g + hf * 256
                    if z0 + 256 <= ZC:
                        og = out[z0:z0 + 256, :].rearrange(
                            "(dt p) ij -> p dt ij", dt=2)
                        nc.sync.dma_start(
                            og, k_sb[:, hf * 2 * IJ:(hf + 1) * 2 * IJ]
                            .rearrange("p (dt ij) -> p dt ij", dt=2))
                    else:
                        for dt in (2 * hf, 2 * hf + 1):
                            zt = 512 * g + dt * 128
                            if zt >= ZC:
                                break
                            rows = min(128, ZC - zt)
                            nc.sync.dma_start(
                                out[zt:zt + rows, :],
                                k_sb[0:rows, dt * IJ:(dt + 1) * IJ])

                # Software-pipelined emission (v5 structure).  PE order:
                # [pair transposes (odd g)] [h_g] [main g-1: dt0,dt1] [R_g]
                # [main g-1: dt2,dt3]; the tps bounce for g heads the
                # iteration.
                emit_pair(0)
                prev = None      # (bstk) of group g-1
                tsl = emit_tpscopy(0)
                for g in range(NG):
                    h_sb = emit_front(g, tsl)
                    if prev is not None:
                        pg, pb = prev
                        pk_sb = opool.tile([128, TB * IJ], BF16, tag="ksb")
                        t01 = emit_main(pg, pb, (0, 1))
                        emit_copy(pk_sb, 0, t01[0])
                        r_ps = emit_r(g, h_sb)
                        emit_copy(pk_sb, 1, t01[1])
                        emit_store(pg, pk_sb, 0)
                        t23 = emit_main(pg, pb, (2, 3))
                        emit_copy(pk_sb, 2, t23[0])
                        emit_copy(pk_sb, 3, t23[1])
                        emit_store(pg, pk_sb, 1)
                    else:
                        r_ps = emit_r(g, h_sb)
                    prev = (g, emit_bmult(g, tsl, r_ps))
                    if g % 2 == 1 and g // 2 + 1 < NPAIR:
                        emit_pair(g // 2 + 1)
                    if g + 1 < NG:
                        tsl = emit_tpscopy(g + 1)
                pg, pb = prev
                pk_sb = opool.tile([128, TB * IJ], BF16, tag="ksb")
                for dt, kt in zip(range(TB), emit_main(pg, pb, (0, 1, 2, 3))):
                    emit_copy(pk_sb, dt, kt)
                emit_store(pg, pk_sb, 0)
                emit_store(pg, pk_sb, 1)

    nc.compile()
    return nc


def _get_program():
    if "nc" not in _CACHE:
        _CACHE["nc"] = _build_program()
    return _CACHE["nc"]


def _host_prep(r, W1, b1, W2, b2, cg, ylm_mix, rf_mix, norm_coef):
    r = np.asarray(r, dtype=np.float32)
    W1 = np.asarray(W1, dtype=np.float32)
    b1 = np.asarray(b1, dtype=np.float32)
    W2 = np.asarray(W2, dtype=np.float32)
    b2 = np.asarray(b2, dtype=np.float32)
    cg = np.asarray(cg, dtype=np.float32)
    ylm_mix = np.asarray(ylm_mix, dtype=np.float32)
    rf_mix = np.asarray(rf_mix, dtype=np.float32)
    norm_coef = np.asarray(norm_coef, dtype=np.float32)

    # constant folding: M64[p*9+l, ij] = sum_k rf[k,p] ylm_s[k,l] cg[k,ij] * nc0
    ylm_s = ylm_mix.astype(np.float64) * YLM_SCALE[None, :]
    wkp = rf_mix.astype(np.float64)[:, :, None] * ylm_s[:, None, :]  # [K,p,l]
    mcore = np.einsum("kq,kj->qj", wkp.reshape(KDIM, 54),
                      cg.astype(np.float64).reshape(KDIM, IJ))
    nc0 = norm_coef.astype(np.float64)[:, :, 0].reshape(1, IJ)
    m64 = np.zeros((NQ, IJ), dtype=np.float32)
    m64[0:54] = (mcore * nc0).astype(np.float32)

    import ml_dtypes
    w1e2 = np.zeros((W, 128), dtype=ml_dtypes.bfloat16)
    w1e2[63, :] = W1[0].astype(ml_dtypes.bfloat16)

    w2e64 = np.zeros((H, NQ), dtype=np.float32)
    w2e64[:, 0:54] = np.repeat(W2, 9, axis=1)

    addv = np.zeros((NQ, 1), dtype=np.float32)
    addv[0:54, 0] = np.repeat(b2, 9)
    addv[54:63, 0] = 1.0

    shared = {
        "m64d": m64,
        "w1e2d": w1e2,
        "w2e64d": w2e64,
        "b1d": np.ascontiguousarray(b1.reshape(H, 1)),
        "addvd": addv,
        "identd": np.eye(128, dtype=ml_dtypes.bfloat16),
    }

    in_maps = []
    for c in range(NCORES):
        rs = r[c * ZC:(c + 1) * ZC]
        rp = np.empty((ZC_PAD, 3), dtype=np.float32)
        rp[:ZC] = rs
        rp[ZC:] = np.array([1.0, 0.0, 0.0], dtype=np.float32)
        rpl = rp.reshape(T, 128, 3).transpose(1, 2, 0).reshape(128, 3 * T)
        m = dict(shared)
        m["rpl"] = np.ascontiguousarray(rpl)
        in_maps.append(m)
    return in_maps


def _run_device(in_maps, trace=False, **kw):
    nc = _get_program()
    return run_bass_kernel_spmd(nc, in_maps, core_ids=list(range(NCORES)),
                                trace=trace, **kw)


def kernel(r, W1, b1, W2, b2, cg, ylm_mix, rf_mix, norm_coef):
    r = np.asarray(r, dtype=np.float32)
    norm_coef_f = np.asarray(norm_coef, dtype=np.float32)
    in_maps = _host_prep(r, W1, b1, W2, b2, cg, ylm_mix, rf_mix, norm_coef_f)
    res = _run_device(in_maps)
    out = np.concatenate(
        [np.asarray(res.results[c]["out"]).astype(np.float32)
         for c in range(NCORES)], axis=0)

    # points with exactly zero radius use norm_coef[..., 1] instead of [..., 0]
    x, y, z = r[:, 0], r[:, 1], r[:, 2]
    r2 = (x * x + y * y) + z * z
    zero = r2 == np.float32(0.0)
    if np.any(zero):
        scale = (norm_coef_f[:, :, 1].astype(np.float64)
                 / norm_coef_f[:, :, 0].astype(np.float64)).reshape(1, IJ)
        out[zero] = (out[zero].astype(np.float64) * scale).astype(np.float32)

    return out.reshape(Z, DO, DI)


# revision 38
# speedup vs baseline: 1.8759x; 1.0628x over previous
"""Trainium2 Bass kernel for the gnn_message_passing problem.

Math refactor: the reference computes
    kernel[z,i,j] = einsum('zk,kij->zij', Rk*Yk, cg) * nc0[i,j]
with Rk = R @ rf_mix.T (rank 6) and Yk = Y.T @ ylm_mix.T (rank 9).
Fold the K=1024 contraction into a host-precomputed constant
    M[p*9+l, ij] = sum_k rf[k,p]*ylm_s[k,l]*cg[k,ij] * nc0[ij]
so each point only needs B[z, pl] = (R[z,p]+b2[p]) * Y'[z,l] contracted
against M - a k=64 fp32r matmul per 128-point tile.  The output is
written to DRAM in bf16 (halving the dominant HBM traffic) and widened
to fp32 on the host; bf16 keeps the max relative error ~2e-3, well
inside the 2e-2 gate.

Device pipeline per 512-point group (4 tiles of 128):
  PE   : per-pair bf16 channel transposes, radial-MLP hidden matmul,
         radial matmul, 8 k=64 output matmuls (fp32r, N=512)
  ACT  : relu (bias b1 fused), ~1.6 output-tile bf16 copies
  DVE  : tps bounce (2x packed), fused B build ((R+b2)*Y via one
         scalar_tensor_tensor), ~2.4 output-tile bf16 copies
  DMA  : two 0.5 MB output stores (fire as soon as their tiles copy)

The channel planes hold the 9 SH ratios pre-replicated across the 6
radial paths (plus plain ratios and radii), so the per-group transpose
directly yields the [64, points] operand layout and no per-group
replication work is needed.

Distribution: data-parallel over z across 8 NeuronCores; constants
replicated. Full inputs in, full output out.
"""

import numpy as np

import concourse.bass as bass
import concourse.tile as tile
from concourse import bacc, mybir
from concourse.bass_utils import run_bass_kernel_spmd

F32 = mybir.dt.float32
F32R = mybir.dt.float32r
BF16 = mybir.dt.bfloat16
ALU = mybir.AluOpType
ACTF = mybir.ActivationFunctionType

# The transposed-channel path runs in bf16 (2^-9 relative rounding; the
# rel-err budget is 2e-2): PE transposes at 1 cycle/row, the pair-transpose
# PSUM tile is a single bank, and the per-group tps bounce uses the DVE 2x
# packed mode.  The walrus BIR verifier requires every FP32r matmult operand
# to be PRODUCED by an engine op with f32r output dtype (a DMA of f32 bits
# does not count), so M and W2 are staged through a one-time engine copy.

# Problem shape (hardcoded per contract)
Z, KDIM, DO, DI, NPATH, H = 100000, 1024, 32, 32, 6, 128
IJ = DO * DI                      # 1024
NCORES = 8
ZC = Z // NCORES                  # 12500 points per core
T = 100                           # point tiles of 128 -> ZC padded to 12800
ZC_PAD = 128 * T
TB = 4                            # tiles per group
NG = T // TB                      # 25 groups
W = 64                            # transposed channel rows: 54 rep + 9 plain + radii
NQ = 64                           # B-stack rows

# Real spherical harmonic constants (l=0,1,2), folded into M host-side
C0 = 0.28209479177387814
C1 = 0.4886025119029199
C2A = 1.0925484305920792
C2B = 0.31539156525252005
C2C = 0.5462742152960396
YLM_SCALE = np.array([C0, C1, C1, C1, C2A, C2A, C2B, C2A, C2C], dtype=np.float64)

_CACHE = {}


def _build_program():
    nc = bacc.Bacc("TRN2", target_bir_lowering=False, debug=False,
                   num_devices=NCORES)

    # ---- per-core DRAM I/O ----
    rpl = nc.dram_tensor("rpl", [128, 3 * T], F32, kind="ExternalInput").ap()
    m64d = nc.dram_tensor("m64d", [NQ, IJ], F32, kind="ExternalInput").ap()
    w1e2d = nc.dram_tensor("w1e2d", [W, 128], BF16, kind="ExternalInput").ap()
    w2e64d = nc.dram_tensor("w2e64d", [H, NQ], F32, kind="ExternalInput").ap()
    b1d = nc.dram_tensor("b1d", [H, 1], F32, kind="ExternalInput").ap()
    addvd = nc.dram_tensor("addvd", [NQ, 1], F32, kind="ExternalInput").ap()
    identd = nc.dram_tensor("identd", [128, 128], BF16, kind="ExternalInput").ap()
    out = nc.dram_tensor("out", [ZC, IJ], BF16, kind="ExternalOutput").ap()

    with tile.TileContext(nc) as tc:
        with tc.tile_pool(name="const", bufs=1) as cpool:
            # ---- resident constants (rpl first: everything downstream
            # of the monomial prologue waits on it) ----
            rpl_sb = cpool.tile([128, 3 * T], F32)
            nc.sync.dma_start(rpl_sb[:], rpl[:])
            m_stage = cpool.tile([NQ, IJ], F32)
            nc.sync.dma_start(m_stage[:], m64d[:])
            m_sb = cpool.tile([NQ, IJ], F32R)
            w1_sb = cpool.tile([W, 128], BF16)
            nc.sync.dma_start(w1_sb[:], w1e2d[:])
            w2_stage = cpool.tile([H, NQ], F32)
            nc.sync.dma_start(w2_stage[:], w2e64d[:])
            w2_sb = cpool.tile([H, NQ], F32R)
            b1_sb = cpool.tile([H, 1], F32)
            nc.sync.dma_start(b1_sb[:], b1d[:])
            addv_sb = cpool.tile([NQ, 1], F32)
            nc.sync.dma_start(addv_sb[:], addvd[:])
            id_sb = cpool.tile([128, 128], BF16)
            nc.sync.dma_start(id_sb[:], identd[:])

            x = rpl_sb[:, 0:T]
            y = rpl_sb[:, T:2 * T]
            z = rpl_sb[:, 2 * T:3 * T]

            # =========================================================
            # Prologue: channel planes.  chan col layout per tile t:
            #   t*64 + p*9 + l : ratio_l (replicated per path p)
            #   t*64 + 54 + l  : ratio_l (plain; l=0 is the ones channel)
            #   t*64 + 63      : radii
            # =========================================================
            chan = cpool.tile([128, W * T], BF16)
            aux = cpool.tile([128, 12 * T], F32)

            def ax(i):
                return aux[:, i * T:(i + 1) * T]

            (xx, yy, zz, r2, safe, inv2, invr, tmp,
             tmp2, tmp3, tmp4, tmp5) = (ax(i) for i in range(12))
            chv = chan[:].rearrange("p (t c) -> p t c", c=W)

            def pl(j):   # plain channel column j (one col per tile)
                return chv[:, :, j]

            nc.vector.tensor_tensor(xx, x, x, ALU.mult)
            nc.vector.tensor_tensor(yy, y, y, ALU.mult)
            nc.vector.tensor_tensor(zz, z, z, ALU.mult)
            nc.vector.tensor_tensor(tmp, xx, yy, ALU.add)
            nc.vector.tensor_tensor(r2, tmp, zz, ALU.add)
            # guard r2 == 0 exactly like the reference's safe_r2
            nc.vector.tensor_scalar(safe, r2, 0.0, None, ALU.is_equal)
            nc.vector.tensor_tensor(safe, r2, safe, ALU.add)
            nc.vector.reciprocal(inv2, safe)              # 1/safe_r2
            nc.scalar.sqrt(invr, inv2)                    # 1/safe_r (~7e-6 rel)

            # products that only need x/y/z go to Pool in parallel with
            # the DVE chain (GPSIMD has no PSUM port but SBUF ops are fine)
            nc.vector.memset(pl(54), 1.0)                 # ones (l=0)
            nc.vector.tensor_tensor(pl(55), y, invr, ALU.mult)
            nc.vector.tensor_tensor(pl(56), z, invr, ALU.mult)
            nc.vector.tensor_tensor(pl(57), x, invr, ALU.mult)
            nc.vector.tensor_tensor(tmp2, x, y, ALU.mult)
            nc.vector.tensor_tensor(pl(58), tmp2, inv2, ALU.mult)
            nc.vector.tensor_tensor(tmp3, y, z, ALU.mult)
            nc.vector.tensor_tensor(pl(59), tmp3, inv2, ALU.mult)
            nc.vector.scalar_tensor_tensor(tmp, zz, 3.0, r2, ALU.mult,
                                           ALU.subtract)
            nc.vector.tensor_tensor(pl(60), tmp, inv2, ALU.mult)
            nc.vector.tensor_tensor(tmp4, x, z, ALU.mult)
            nc.vector.tensor_tensor(pl(61), tmp4, inv2, ALU.mult)
            nc.vector.tensor_tensor(tmp5, xx, yy, ALU.subtract)
            nc.vector.tensor_tensor(pl(62), tmp5, inv2, ALU.mult)
            nc.vector.tensor_tensor(pl(63), r2, invr, ALU.mult)   # radii

            # replicate the 9 plain ratios into the 6 path blocks; the
            # first pair's 8 tiles go first so its transposes start while
            # the remaining 92 tiles replicate, and the one-time f32r
            # weight staging copies slot in between (off the critical path)
            chv8 = chan[:, 0:8 * W].rearrange("p (t c) -> p t c", c=W)
            chv92 = chan[:, 8 * W:].rearrange("p (t c) -> p t c", c=W)
            for p in range(NPATH):
                nc.vector.tensor_copy(chv8[:, :, p * 9:p * 9 + 9],
                                      chv8[:, :, 54:63])
            nc.vector.tensor_copy(w2_sb[:], w2_stage[:])
            nc.vector.tensor_copy(m_sb[:], m_stage[:])
            for p in range(NPATH):
                nc.vector.tensor_copy(chv92[:, :, p * 9:p * 9 + 9],
                                      chv92[:, :, 54:63])

            # =========================================================
            # Main loop: 25 groups of 4 tiles; transposes batched in
            # pairs of groups (8 tiles -> [64, 1024] PSUM -> DMA bounce)
            # =========================================================
            NPAIR = (NG + 1) // 2     # 13 (last pair is half-size)

            with tc.tile_pool(name="shps", bufs=1, space="PSUM") as shpool, \
                 tc.tile_pool(name="hps", bufs=1, space="PSUM") as hpool, \
                 tc.tile_pool(name="kps", bufs=3, space="PSUM") as kpool, \
                 tc.tile_pool(name="tsb", bufs=2) as tspool, \
                 tc.tile_pool(name="work", bufs=2) as wpool, \
                 tc.tile_pool(name="kout", bufs=3) as opool:

                tp_ps = {}
                tps_sb = {}
                big_ps = {}

                def emit_pair(pr):
                    # transpose 8 (or 4) tiles' channels into PSUM.  The
                    # pair tile shares one physical PSUM bank with r_ps:
                    # the f32 radial output lives on partitions 0-63
                    # (matmul PSUM writes must start at partition 0), the
                    # bf16 transposes on partitions 64-127 (transpose
                    # writes at base 64 pass the walrus ISA check).
                    t0 = pr * 2 * TB
                    ntile = min(2 * TB, T - t0)
                    big = shpool.tile([128, 512], F32, tag="big",
                                      name="big_ps")
                    tp = big[NQ:128, :].bitcast(BF16)
                    for i in range(ntile):
                        nc.tensor.transpose(
                            tp[:, i * 128:(i + 1) * 128],
                            chan[:, (t0 + i) * W:(t0 + i + 1) * W],
                            id_sb[:])
                    big_ps[pr] = big
                    tp_ps[pr] = tp
                    tps_sb[pr] = tspool.tile([W, 2 * TB * 128], BF16,
                                             tag="tsb", name="tps_sb")

                def emit_tpscopy(g):
                    # bounce group g's tps half PSUM->SBUF (DVE 2x packed)
                    pr, hf = g // 2, g % 2
                    tsl = tps_sb[pr][:, hf * 512:hf * 512 + 512]
                    nc.vector.tensor_copy(
                        tsl, tp_ps[pr][:, hf * 512:hf * 512 + 512])
                    return tsl

                def emit_front(g, tsl):
                    # radial MLP hidden layer + relu
                    h_ps = hpool.tile([128, 512], F32, tag="hps")
                    nc.tensor.matmul(h_ps[:], w1_sb[:], tsl,
                                     start=True, stop=True)
                    h_sb = wpool.tile([128, 512], F32R, tag="hsb")
                    nc.scalar.activation(h_sb[:], h_ps[:], ACTF.Relu,
                                         bias=b1_sb[:])
                    return h_sb

                def emit_r(g, h_sb):
                    r_ps = big_ps[g // 2][0:NQ, :]
                    nc.tensor.matmul(r_ps, w2_sb[:], h_sb[:],
                                     start=True, stop=True)
                    return r_ps

                def emit_bmult(g, tsl, r_ps):
                    # fused B build: bstk = (R + addv) * Y  (one DVE op)
                    bstk = wpool.tile([NQ, 512], F32R, tag="bstk")
                    nc.vector.scalar_tensor_tensor(
                        bstk[:], r_ps, addv_sb[:], tsl,
                        ALU.add, ALU.mult)
                    return bstk

                def emit_main(g, bstk, dts):
                    # k=64 output matmuls for tiles `dts` of group g
                    tiles = []
                    for dt in dts:
                        k_ps = kpool.tile([128, IJ], F32, tag="kps")
                        for half in range(2):
                            nc.tensor.matmul(
                                k_ps[:, half * 512:(half + 1) * 512],
                                bstk[:, dt * 128:(dt + 1) * 128],
                                m_sb[:, half * 512:(half + 1) * 512],
                                start=True, stop=True)
                        tiles.append(k_ps)
                    return tiles

                def emit_copy(k_sb, dt, k_ps):
                    # PSUM f32 -> SBUF bf16; dt0 split across both engines
                    # so its k-PSUM slot frees early (dt2 reuses it)
                    dst = k_sb[:, dt * IJ:(dt + 1) * IJ]
                    if dt == 0:
                        nc.vector.tensor_copy(dst[:, 0:712], k_ps[:, 0:712])
                        nc.scalar.copy(dst[:, 712:IJ], k_ps[:, 712:IJ])
                    elif dt == 2:
                        nc.vector.tensor_copy(dst[:], k_ps[:])
                    else:
                        nc.scalar.copy(dst[:], k_ps[:])

                def emit_store(g, k_sb, hf):
                    # store half hf (2 tiles = 256 points) of group g; the
                    # first half is issued as soon as dt0/dt1 are copied so
                    # the DMA engines never sit idle waiting for dt3
                    z0 = 512 * g + hf * 256
                    if z0 + 256 <= ZC:
                        og = out[z0:z0 + 256, :].rearrange(
                            "(dt p) ij -> p dt ij", dt=2)
                        nc.sync.dma_start(
                            og, k_sb[:, hf * 2 * IJ:(hf + 1) * 2 * IJ]
                            .rearrange("p (dt ij) -> p dt ij", dt=2))
                    else:
                        for dt in (2 * hf, 2 * hf + 1):
                            zt = 512 * g + dt * 128
                            if zt >= ZC:
                                break
                            rows = min(128, ZC - zt)
                            nc.sync.dma_start(
                                out[zt:zt + rows, :],
                                k_sb[0:rows, dt * IJ:(dt + 1) * IJ])

                # Software-pipelined emission (v5 structure).  PE order:
                # [pair transposes (odd g)] [h_g] [main g-1: dt0,dt1] [R_g]
                # [main g-1: dt2,dt3]; the tps bounce for g heads the
                # iteration.
                emit_pair(0)
                prev = None      # (bstk) of group g-1
                tsl = emit_tpscopy(0)
                for g in range(NG):
                    h_sb = emit_front(g, tsl)
                    if prev is not None:
                        pg, pb = prev
                        pk_sb = opool.tile([128, TB * IJ], BF16, tag="ksb")
                        t01 = emit_main(pg, pb, (0, 1))
                        emit_copy(pk_sb, 0, t01[0])
                        r_ps = emit_r(g, h_sb)
                        emit_copy(pk_sb, 1, t01[1])
                        emit_store(pg, pk_sb, 0)
                        t23 = emit_main(pg, pb, (2, 3))
                        emit_copy(pk_sb, 2, t23[0])
                        emit_copy(pk_sb, 3, t23[1])
                        emit_store(pg, pk_sb, 1)
                    else:
                        r_ps = emit_r(g, h_sb)
                    prev = (g, emit_bmult(g, tsl, r_ps))
                    if g % 2 == 1 and g // 2 + 1 < NPAIR:
                        emit_pair(g // 2 + 1)
                    if g + 1 < NG:
                        tsl = emit_tpscopy(g + 1)
                pg, pb = prev
                pk_sb = opool.tile([128, TB * IJ], BF16, tag="ksb")
                for dt, kt in zip(range(TB), emit_main(pg, pb, (0, 1, 2, 3))):
                    emit_copy(pk_sb, dt, kt)
                emit_store(pg, pk_sb, 0)
                emit_store(pg, pk_sb, 1)

    nc.compile()
    return nc


def _get_program():
    if "nc" not in _CACHE:
        _CACHE["nc"] = _build_program()
    return _CACHE["nc"]


def _host_prep(r, W1, b1, W2, b2, cg, ylm_mix, rf_mix, norm_coef):
    r = np.asarray(r, dtype=np.float32)
    W1 = np.asarray(W1, dtype=np.float32)
    b1 = np.asarray(b1, dtype=np.float32)
    W2 = np.asarray(W2, dtype=np.float32)
    b2 = np.asarray(b2, dtype=np.float32)
    cg = np.asarray(cg, dtype=np.float32)
    ylm_mix = np.asarray(ylm_mix, dtype=np.float32)
    rf_mix = np.asarray(rf_mix, dtype=np.float32)
    norm_coef = np.asarray(norm_coef, dtype=np.float32)

    # constant folding: M64[p*9+l, ij] = sum_k rf[k,p] ylm_s[k,l] cg[k,ij] * nc0
    ylm_s = ylm_mix.astype(np.float64) * YLM_SCALE[None, :]
    wkp = rf_mix.astype(np.float64)[:, :, None] * ylm_s[:, None, :]  # [K,p,l]
    mcore = np.einsum("kq,kj->qj", wkp.reshape(KDIM, 54),
                      cg.astype(np.float64).reshape(KDIM, IJ))
    nc0 = norm_coef.astype(np.float64)[:, :, 0].reshape(1, IJ)
    m64 = np.zeros((NQ, IJ), dtype=np.float32)
    m64[0:54] = (mcore * nc0).astype(np.float32)

    import ml_dtypes
    w1e2 = np.zeros((W, 128), dtype=ml_dtypes.bfloat16)
    w1e2[63, :] = W1[0].astype(ml_dtypes.bfloat16)

    w2e64 = np.zeros((H, NQ), dtype=np.float32)
    w2e64[:, 0:54] = np.repeat(W2, 9, axis=1)

    addv = np.zeros((NQ, 1), dtype=np.float32)
    addv[0:54, 0] = np.repeat(b2, 9)
    addv[54:63, 0] = 1.0

    shared = {
        "m64d": m64,
        "w1e2d": w1e2,
        "w2e64d": w2e64,
        "b1d": np.ascontiguousarray(b1.reshape(H, 1)),
        "addvd": addv,
        "identd": np.eye(128, dtype=ml_dtypes.bfloat16),
    }

    in_maps = []
    for c in range(NCORES):
        rs = r[c * ZC:(c + 1) * ZC]
        rp = np.empty((ZC_PAD, 3), dtype=np.float32)
        rp[:ZC] = rs
        rp[ZC:] = np.array([1.0, 0.0, 0.0], dtype=np.float32)
        rpl = rp.reshape(T, 128, 3).transpose(1, 2, 0).reshape(128, 3 * T)
        m = dict(shared)
        m["rpl"] = np.ascontiguousarray(rpl)
        in_maps.append(m)
    return in_maps


def _run_device(in_maps, trace=False, **kw):
    nc = _get_program()
    return run_bass_kernel_spmd(nc, in_maps, core_ids=list(range(NCORES)),
                                trace=trace, **kw)


def kernel(r, W1, b1, W2, b2, cg, ylm_mix, rf_mix, norm_coef):
    r = np.asarray(r, dtype=np.float32)
    norm_coef_f = np.asarray(norm_coef, dtype=np.float32)
    in_maps = _host_prep(r, W1, b1, W2, b2, cg, ylm_mix, rf_mix, norm_coef_f)
    res = _run_device(in_maps)
    out = np.concatenate(
        [np.asarray(res.results[c]["out"]).astype(np.float32)
         for c in range(NCORES)], axis=0)

    # points with exactly zero radius use norm_coef[..., 1] instead of [..., 0]
    x, y, z = r[:, 0], r[:, 1], r[:, 2]
    r2 = (x * x + y * y) + z * z
    zero = r2 == np.float32(0.0)
    if np.any(zero):
        scale = (norm_coef_f[:, :, 1].astype(np.float64)
                 / norm_coef_f[:, :, 0].astype(np.float64)).reshape(1, IJ)
        out[zero] = (out[zero].astype(np.float64) * scale).astype(np.float32)

    return out.reshape(Z, DO, DI)


# revision 42
# speedup vs baseline: 2.0609x; 1.0986x over previous
"""Trainium2 Bass kernel for the gnn_message_passing problem.

Math refactor: the reference computes
    kernel[z,i,j] = einsum('zk,kij->zij', Rk*Yk, cg) * nc0[i,j]
with Rk = R @ rf_mix.T (rank 6) and Yk = Y.T @ ylm_mix.T (rank 9).
Fold the K=1024 contraction into a host-precomputed constant
    M[p*9+l, ij] = sum_k rf[k,p]*ylm_s[k,l]*cg[k,ij] * nc0[ij]
so each point only needs B[z, pl] = (R[z,p]+b2[p]) * Y'[z,l] contracted
against M - a k=64 fp32r matmul per 128-point tile.  The output is
written to DRAM in bf16 (halving the dominant HBM traffic) and widened
to fp32 on the host; bf16 keeps the max relative error ~2e-3, well
inside the 2e-2 gate.

Device pipeline per 512-point group (4 tiles of 128):
  PE   : per-pair bf16 channel transposes, radial-MLP hidden matmul,
         radial matmul, 8 k=64 output matmuls (fp32r, N=512)
  ACT  : relu (bias b1 fused), ~1.6 output-tile bf16 copies
  DVE  : tps bounce (2x packed), fused B build ((R+b2)*Y via one
         scalar_tensor_tensor), ~2.4 output-tile bf16 copies
  DMA  : two 0.5 MB output stores (fire as soon as their tiles copy)

The channel planes hold the 9 SH ratios pre-replicated across the 6
radial paths (plus plain ratios and radii), so the per-group transpose
directly yields the [64, points] operand layout and no per-group
replication work is needed.

Distribution: data-parallel over z across 8 NeuronCores; constants
replicated. Full inputs in, full output out.
"""

import numpy as np

import concourse.bass as bass
import concourse.tile as tile
from concourse import bacc, mybir
from concourse.bass_utils import run_bass_kernel_spmd

F32 = mybir.dt.float32
F32R = mybir.dt.float32r
BF16 = mybir.dt.bfloat16
ALU = mybir.AluOpType
ACTF = mybir.ActivationFunctionType

# The transposed-channel path runs in bf16 (2^-9 relative rounding; the
# rel-err budget is 2e-2): PE transposes at 1 cycle/row, the pair-transpose
# PSUM tile is a single bank, and the per-group tps bounce uses the DVE 2x
# packed mode.  The walrus BIR verifier requires every FP32r matmult operand
# to be PRODUCED by an engine op with f32r output dtype (a DMA of f32 bits
# does not count), so M and W2 are staged through a one-time engine copy.

# Problem shape (hardcoded per contract)
Z, KDIM, DO, DI, NPATH, H = 100000, 1024, 32, 32, 6, 128
IJ = DO * DI                      # 1024
NCORES = 8
ZC = Z // NCORES                  # 12500 points per core
T = 100                           # point tiles of 128 -> ZC padded to 12800
ZC_PAD = 128 * T
TB = 4                            # tiles per group
NG = T // TB                      # 25 groups
W = 64                            # transposed channel rows: 54 rep + 9 plain + radii
NQ = 64                           # B-stack rows

# Real spherical harmonic constants (l=0,1,2), folded into M host-side
C0 = 0.28209479177387814
C1 = 0.4886025119029199
C2A = 1.0925484305920792
C2B = 0.31539156525252005
C2C = 0.5462742152960396
YLM_SCALE = np.array([C0, C1, C1, C1, C2A, C2A, C2B, C2A, C2C], dtype=np.float64)

_CACHE = {}


def _build_program():
    nc = bacc.Bacc("TRN2", target_bir_lowering=False, debug=False,
                   num_devices=NCORES)

    # ---- per-core DRAM I/O ----
    rpl = nc.dram_tensor("rpl", [128, 3 * T], F32, kind="ExternalInput").ap()
    m64d = nc.dram_tensor("m64d", [NQ, IJ], F32, kind="ExternalInput").ap()
    w1e2d = nc.dram_tensor("w1e2d", [W, 128], BF16, kind="ExternalInput").ap()
    w2e64d = nc.dram_tensor("w2e64d", [H, NQ], F32, kind="ExternalInput").ap()
    b1d = nc.dram_tensor("b1d", [H, 1], F32, kind="ExternalInput").ap()
    addvd = nc.dram_tensor("addvd", [NQ, 1], F32, kind="ExternalInput").ap()
    identd = nc.dram_tensor("identd", [128, 128], BF16, kind="ExternalInput").ap()
    out = nc.dram_tensor("out", [ZC, IJ], BF16, kind="ExternalOutput").ap()

    with tile.TileContext(nc) as tc:
        with tc.tile_pool(name="const", bufs=1) as cpool:
            # ---- resident constants (rpl first: everything downstream
            # of the monomial prologue waits on it) ----
            rpl_sb = cpool.tile([128, 3 * T], F32)
            nc.sync.dma_start(rpl_sb[:], rpl[:])
            m_stage = cpool.tile([NQ, IJ], F32)
            nc.sync.dma_start(m_stage[:], m64d[:])
            m_sb = cpool.tile([NQ, IJ], F32R)
            w1_sb = cpool.tile([W, 128], BF16)
            nc.sync.dma_start(w1_sb[:], w1e2d[:])
            w2_stage = cpool.tile([H, NQ], F32)
            nc.sync.dma_start(w2_stage[:], w2e64d[:])
            w2_sb = cpool.tile([H, NQ], F32R)
            b1_sb = cpool.tile([H, 1], F32)
            nc.sync.dma_start(b1_sb[:], b1d[:])
            addv_sb = cpool.tile([NQ, 1], F32)
            nc.sync.dma_start(addv_sb[:], addvd[:])
            id_sb = cpool.tile([128, 128], BF16)
            nc.sync.dma_start(id_sb[:], identd[:])

            x = rpl_sb[:, 0:T]
            y = rpl_sb[:, T:2 * T]
            z = rpl_sb[:, 2 * T:3 * T]

            # =========================================================
            # Prologue: channel planes.  chan col layout per tile t:
            #   t*64 + p*9 + l : ratio_l (replicated per path p)
            #   t*64 + 54 + l  : ratio_l (plain; l=0 is the ones channel)
            #   t*64 + 63      : radii
            # =========================================================
            chan = cpool.tile([128, W * T], BF16)
            aux = cpool.tile([128, 12 * T], F32)

            def ax(i):
                return aux[:, i * T:(i + 1) * T]

            (xx, yy, zz, r2, safe, inv2, invr, tmp,
             tmp2, tmp3, tmp4, tmp5) = (ax(i) for i in range(12))
            chv = chan[:].rearrange("p (t c) -> p t c", c=W)

            def pl(j):   # plain channel column j (one col per tile)
                return chv[:, :, j]

            nc.vector.tensor_tensor(xx, x, x, ALU.mult)
            nc.vector.tensor_tensor(yy, y, y, ALU.mult)
            nc.vector.tensor_tensor(zz, z, z, ALU.mult)
            nc.vector.tensor_tensor(tmp, xx, yy, ALU.add)
            nc.vector.tensor_tensor(r2, tmp, zz, ALU.add)
            # guard r2 == 0 exactly like the reference's safe_r2
            nc.vector.tensor_scalar(safe, r2, 0.0, None, ALU.is_equal)
            nc.vector.tensor_tensor(safe, r2, safe, ALU.add)
            nc.vector.reciprocal(inv2, safe)              # 1/safe_r2
            nc.scalar.sqrt(invr, inv2)                    # 1/safe_r (~7e-6 rel)

            # products that only need x/y/z go to Pool in parallel with
            # the DVE chain (GPSIMD has no PSUM port but SBUF ops are fine)
            nc.vector.memset(pl(54), 1.0)                 # ones (l=0)
            nc.vector.tensor_tensor(pl(55), y, invr, ALU.mult)
            nc.vector.tensor_tensor(pl(56), z, invr, ALU.mult)
            nc.vector.tensor_tensor(pl(57), x, invr, ALU.mult)
            nc.vector.tensor_tensor(tmp2, x, y, ALU.mult)
            nc.vector.tensor_tensor(pl(58), tmp2, inv2, ALU.mult)
            nc.vector.tensor_tensor(tmp3, y, z, ALU.mult)
            nc.vector.tensor_tensor(pl(59), tmp3, inv2, ALU.mult)
            nc.vector.scalar_tensor_tensor(tmp, zz, 3.0, r2, ALU.mult,
                                           ALU.subtract)
            nc.vector.tensor_tensor(pl(60), tmp, inv2, ALU.mult)
            nc.vector.tensor_tensor(tmp4, x, z, ALU.mult)
            nc.vector.tensor_tensor(pl(61), tmp4, inv2, ALU.mult)
            nc.vector.tensor_tensor(tmp5, xx, yy, ALU.subtract)
            nc.vector.tensor_tensor(pl(62), tmp5, inv2, ALU.mult)
            nc.vector.tensor_tensor(pl(63), r2, invr, ALU.mult)   # radii

            # replicate the 9 plain ratios into the 6 path blocks; the
            # first pair's 8 tiles go first so its transposes start while
            # the remaining 92 tiles replicate, and the one-time f32r
            # weight staging copies slot in between (off the critical path)
            chv8 = chan[:, 0:8 * W].rearrange("p (t c) -> p t c", c=W)
            chv92 = chan[:, 8 * W:].rearrange("p (t c) -> p t c", c=W)
            for p in range(NPATH):
                nc.vector.tensor_copy(chv8[:, :, p * 9:p * 9 + 9],
                                      chv8[:, :, 54:63])
            nc.vector.tensor_copy(w2_sb[:], w2_stage[:])
            nc.vector.tensor_copy(m_sb[:], m_stage[:])
            for p in range(NPATH):
                nc.vector.tensor_copy(chv92[:, :, p * 9:p * 9 + 9],
                                      chv92[:, :, 54:63])

            # =========================================================
            # Main loop: 25 groups of 4 tiles; transposes batched in
            # pairs of groups (8 tiles -> [64, 1024] PSUM -> DMA bounce)
            # =========================================================
            NPAIR = (NG + 1) // 2     # 13 (last pair is half-size)

            with tc.tile_pool(name="shps", bufs=1, space="PSUM") as shpool, \
                 tc.tile_pool(name="hps", bufs=1, space="PSUM") as hpool, \
                 tc.tile_pool(name="kps", bufs=3, space="PSUM") as kpool, \
                 tc.tile_pool(name="tsb", bufs=2) as tspool, \
                 tc.tile_pool(name="work", bufs=2) as wpool, \
                 tc.tile_pool(name="kout", bufs=5) as opool:

                tp_ps = {}
                tps_sb = {}
                big_ps = {}

                def emit_pair(pr, half=None):
                    # transpose tiles of pair pr into PSUM (half=0: tiles
                    # 0-3, half=1: tiles 4-7, None: all).  The pair tile
                    # shares one physical PSUM bank with r_ps: the f32
                    # radial output lives on partitions 0-63 (matmul PSUM
                    # writes must start at partition 0), the bf16
                    # transposes on partitions 64-127 (transpose writes at
                    # base 64 pass the walrus ISA check).
                    t0 = pr * 2 * TB
                    ntile = min(2 * TB, T - t0)
                    if pr not in big_ps:
                        big = shpool.tile([128, 512], F32, tag="big",
                                          name="big_ps")
                        big_ps[pr] = big
                        tp_ps[pr] = big[NQ:128, :].bitcast(BF16)
                        tps_sb[pr] = tspool.tile([W, 2 * TB * 128], BF16,
                                                 tag="tsb", name="tps_sb")
                    tp = tp_ps[pr]
                    rng = range(ntile) if half is None else \
                        range(half * TB, min((half + 1) * TB, ntile))
                    for i in rng:
                        nc.tensor.transpose(
                            tp[:, i * 128:(i + 1) * 128],
                            chan[:, (t0 + i) * W:(t0 + i + 1) * W],
                            id_sb[:])

                def emit_tpscopy(g):
                    # bounce group g's tps half PSUM->SBUF (DVE 2x packed)
                    pr, hf = g // 2, g % 2
                    tsl = tps_sb[pr][:, hf * 512:hf * 512 + 512]
                    nc.vector.tensor_copy(
                        tsl, tp_ps[pr][:, hf * 512:hf * 512 + 512])
                    return tsl

                def emit_front(g, tsl):
                    # radial MLP hidden layer + relu
                    h_ps = hpool.tile([128, 512], F32, tag="hps")
                    nc.tensor.matmul(h_ps[:], w1_sb[:], tsl,
                                     start=True, stop=True)
                    h_sb = wpool.tile([128, 512], F32R, tag="hsb")
                    nc.scalar.activation(h_sb[:], h_ps[:], ACTF.Relu,
                                         bias=b1_sb[:])
                    return h_sb

                def emit_r(g, h_sb):
                    r_ps = big_ps[g // 2][0:NQ, :]
                    nc.tensor.matmul(r_ps, w2_sb[:], h_sb[:],
                                     start=True, stop=True)
                    return r_ps

                def emit_bmult(g, tsl, r_ps):
                    # fused B build: bstk = (R + addv) * Y  (one DVE op)
                    bstk = wpool.tile([NQ, 512], F32R, tag="bstk")
                    nc.vector.scalar_tensor_tensor(
                        bstk[:], r_ps, addv_sb[:], tsl,
                        ALU.add, ALU.mult)
                    return bstk

                def emit_main(g, bstk, dts):
                    # k=64 output matmuls for tiles `dts` of group g
                    tiles = []
                    for dt in dts:
                        k_ps = kpool.tile([128, IJ], F32, tag="kps")
                        for half in range(2):
                            nc.tensor.matmul(
                                k_ps[:, half * 512:(half + 1) * 512],
                                bstk[:, dt * 128:(dt + 1) * 128],
                                m_sb[:, half * 512:(half + 1) * 512],
                                start=True, stop=True)
                        tiles.append(k_ps)
                    return tiles

                def emit_copy(k_sb, dt, k_ps):
                    # PSUM f32 -> SBUF bf16; dt0 split across both engines
                    # so its k-PSUM slot frees early (dt2 reuses it)
                    dst = k_sb[:, dt * IJ:(dt + 1) * IJ]
                    if dt == 0:
                        nc.vector.tensor_copy(dst[:, 0:712], k_ps[:, 0:712])
                        nc.scalar.copy(dst[:, 712:IJ], k_ps[:, 712:IJ])
                    elif dt == 2:
                        nc.vector.tensor_copy(dst[:], k_ps[:])
                    else:
                        nc.scalar.copy(dst[:], k_ps[:])

                def emit_store(g, k_sb, hf):
                    # store half hf (2 tiles = 256 points) of group g; the
                    # first half is issued as soon as dt0/dt1 are copied so
                    # the DMA engines never sit idle waiting for dt3
                    z0 = 512 * g + hf * 256
                    if z0 + 256 <= ZC:
                        og = out[z0:z0 + 256, :].rearrange(
                            "(dt p) ij -> p dt ij", dt=2)
                        nc.sync.dma_start(
                            og, k_sb[:, hf * 2 * IJ:(hf + 1) * 2 * IJ]
                            .rearrange("p (dt ij) -> p dt ij", dt=2))
                    else:
                        for dt in (2 * hf, 2 * hf + 1):
                            zt = 512 * g + dt * 128
                            if zt >= ZC:
                                break
                            rows = min(128, ZC - zt)
                            nc.sync.dma_start(
                                out[zt:zt + rows, :],
                                k_sb[0:rows, dt * IJ:(dt + 1) * IJ])

                # Software-pipelined emission (v5 structure).  PE order:
                # [pair transposes (odd g)] [h_g] [main g-1: dt0,dt1] [R_g]
                # [main g-1: dt2,dt3]; the tps bounce for g heads the
                # iteration.
                emit_pair(0)
                prev = None      # (bstk) of group g-1
                tsl = emit_tpscopy(0)
                for g in range(NG):
                    nxt = emit_tpscopy(g + 1) if g + 1 < NG else None
                    h_sb = emit_front(g, tsl)
                    if prev is not None:
                        pg, pb = prev
                        pk_sb = opool.tile([128, TB * IJ], BF16, tag="ksb")
                        t01 = emit_main(pg, pb, (0, 1))
                        emit_copy(pk_sb, 0, t01[0])
                        r_ps = emit_r(g, h_sb)
                        emit_copy(pk_sb, 1, t01[1])
                        emit_store(pg, pk_sb, 0)
                        t23 = emit_main(pg, pb, (2, 3))
                        emit_copy(pk_sb, 2, t23[0])
                        emit_copy(pk_sb, 3, t23[1])
                        emit_store(pg, pk_sb, 1)
                    else:
                        r_ps = emit_r(g, h_sb)
                    prev = (g, emit_bmult(g, tsl, r_ps))
                    if g + 2 < NG:
                        emit_pair((g + 2) // 2, (g + 2) % 2)
                    tsl = nxt
                pg, pb = prev
                pk_sb = opool.tile([128, TB * IJ], BF16, tag="ksb")
                # the final group only has valid points in its first tiles;
                # skip matmuls/copies/stores for pure-padding tiles
                ndt = min(TB, -(-(ZC - 512 * pg) // 128))
                for dt, kt in zip(range(ndt), emit_main(pg, pb, tuple(range(ndt)))):
                    emit_copy(pk_sb, dt, kt)
                emit_store(pg, pk_sb, 0)
                if ndt > 2:
                    emit_store(pg, pk_sb, 1)

    nc.compile()
    return nc


def _get_program():
    if "nc" not in _CACHE:
        _CACHE["nc"] = _build_program()
    return _CACHE["nc"]


def _host_prep(r, W1, b1, W2, b2, cg, ylm_mix, rf_mix, norm_coef):
    r = np.asarray(r, dtype=np.float32)
    W1 = np.asarray(W1, dtype=np.float32)
    b1 = np.asarray(b1, dtype=np.float32)
    W2 = np.asarray(W2, dtype=np.float32)
    b2 = np.asarray(b2, dtype=np.float32)
    cg = np.asarray(cg, dtype=np.float32)
    ylm_mix = np.asarray(ylm_mix, dtype=np.float32)
    rf_mix = np.asarray(rf_mix, dtype=np.float32)
    norm_coef = np.asarray(norm_coef, dtype=np.float32)

    # constant folding: M64[p*9+l, ij] = sum_k rf[k,p] ylm_s[k,l] cg[k,ij] * nc0
    ylm_s = ylm_mix.astype(np.float64) * YLM_SCALE[None, :]
    wkp = rf_mix.astype(np.float64)[:, :, None] * ylm_s[:, None, :]  # [K,p,l]
    mcore = np.einsum("kq,kj->qj", wkp.reshape(KDIM, 54),
                      cg.astype(np.float64).reshape(KDIM, IJ))
    nc0 = norm_coef.astype(np.float64)[:, :, 0].reshape(1, IJ)
    m64 = np.zeros((NQ, IJ), dtype=np.float32)
    m64[0:54] = (mcore * nc0).astype(np.float32)

    import ml_dtypes
    w1e2 = np.zeros((W, 128), dtype=ml_dtypes.bfloat16)
    w1e2[63, :] = W1[0].astype(ml_dtypes.bfloat16)

    w2e64 = np.zeros((H, NQ), dtype=np.float32)
    w2e64[:, 0:54] = np.repeat(W2, 9, axis=1)

    addv = np.zeros((NQ, 1), dtype=np.float32)
    addv[0:54, 0] = np.repeat(b2, 9)
    addv[54:63, 0] = 1.0

    shared = {
        "m64d": m64,
        "w1e2d": w1e2,
        "w2e64d": w2e64,
        "b1d": np.ascontiguousarray(b1.reshape(H, 1)),
        "addvd": addv,
        "identd": np.eye(128, dtype=ml_dtypes.bfloat16),
    }

    in_maps = []
    for c in range(NCORES):
        rs = r[c * ZC:(c + 1) * ZC]
        rp = np.empty((ZC_PAD, 3), dtype=np.float32)
        rp[:ZC] = rs
        rp[ZC:] = np.array([1.0, 0.0, 0.0], dtype=np.float32)
        rpl = rp.reshape(T, 128, 3).transpose(1, 2, 0).reshape(128, 3 * T)
        m = dict(shared)
        m["rpl"] = np.ascontiguousarray(rpl)
        in_maps.append(m)
    return in_maps


def _run_device(in_maps, trace=False, **kw):
    nc = _get_program()
    return run_bass_kernel_spmd(nc, in_maps, core_ids=list(range(NCORES)),
                                trace=trace, **kw)


def kernel(r, W1, b1, W2, b2, cg, ylm_mix, rf_mix, norm_coef):
    r = np.asarray(r, dtype=np.float32)
    norm_coef_f = np.asarray(norm_coef, dtype=np.float32)
    in_maps = _host_prep(r, W1, b1, W2, b2, cg, ylm_mix, rf_mix, norm_coef_f)
    res = _run_device(in_maps)
    out = np.concatenate(
        [np.asarray(res.results[c]["out"]).astype(np.float32)
         for c in range(NCORES)], axis=0)

    # points with exactly zero radius use norm_coef[..., 1] instead of [..., 0]
    x, y, z = r[:, 0], r[:, 1], r[:, 2]
    r2 = (x * x + y * y) + z * z
    zero = r2 == np.float32(0.0)
    if np.any(zero):
        scale = (norm_coef_f[:, :, 1].astype(np.float64)
                 / norm_coef_f[:, :, 0].astype(np.float64)).reshape(1, IJ)
        out[zero] = (out[zero].astype(np.float64) * scale).astype(np.float32)

    return out.reshape(Z, DO, DI)


# revision 53
# speedup vs baseline: 2.0630x; 1.0010x over previous
"""Trainium2 Bass kernel for the gnn_message_passing problem.

Math refactor: the reference computes
    kernel[z,i,j] = einsum('zk,kij->zij', Rk*Yk, cg) * nc0[i,j]
with Rk = R @ rf_mix.T (rank 6) and Yk = Y.T @ ylm_mix.T (rank 9).
Fold the K=1024 contraction into a host-precomputed constant
    M[p*9+l, ij] = sum_k rf[k,p]*ylm_s[k,l]*cg[k,ij] * nc0[ij]
so each point only needs B[z, pl] = (R[z,p]+b2[p]) * Y'[z,l] contracted
against M - a k=64 fp32r matmul per 128-point tile.  The output is
written to DRAM in bf16 (halving the dominant HBM traffic) and widened
to fp32 on the host; bf16 keeps the max relative error ~2e-3, well
inside the 2e-2 gate.

Device pipeline per 512-point group (4 tiles of 128):
  PE   : per-pair bf16 channel transposes, radial-MLP hidden matmul,
         radial matmul, 8 k=64 output matmuls (fp32r, N=512)
  ACT  : relu (bias b1 fused), ~1.6 output-tile bf16 copies
  DVE  : tps bounce (2x packed), fused B build ((R+b2)*Y via one
         scalar_tensor_tensor), ~2.4 output-tile bf16 copies
  DMA  : two 0.5 MB output stores (fire as soon as their tiles copy)

The channel planes hold the 9 SH ratios pre-replicated across the 6
radial paths (plus plain ratios and radii), so the per-group transpose
directly yields the [64, points] operand layout and no per-group
replication work is needed.

Distribution: data-parallel over z across 8 NeuronCores; constants
replicated. Full inputs in, full output out.
"""

import numpy as np

import concourse.bass as bass
import concourse.tile as tile
from concourse import bacc, mybir
from concourse.bass_utils import run_bass_kernel_spmd

F32 = mybir.dt.float32
F32R = mybir.dt.float32r
BF16 = mybir.dt.bfloat16
ALU = mybir.AluOpType
ACTF = mybir.ActivationFunctionType

# The transposed-channel path runs in bf16 (2^-9 relative rounding; the
# rel-err budget is 2e-2): PE transposes at 1 cycle/row, the pair-transpose
# PSUM tile is a single bank, and the per-group tps bounce uses the DVE 2x
# packed mode.  The walrus BIR verifier requires every FP32r matmult operand
# to be PRODUCED by an engine op with f32r output dtype (a DMA of f32 bits
# does not count), so M and W2 are staged through a one-time engine copy.

# Problem shape (hardcoded per contract)
Z, KDIM, DO, DI, NPATH, H = 100000, 1024, 32, 32, 6, 128
IJ = DO * DI                      # 1024
NCORES = 8
ZC = Z // NCORES                  # 12500 points per core
T = 100                           # point tiles of 128 -> ZC padded to 12800
ZC_PAD = 128 * T
TB = 4                            # tiles per group
NG = T // TB                      # 25 groups
W = 64                            # transposed channel rows: 54 rep + 9 plain + radii
NQ = 64                           # B-stack rows

# Real spherical harmonic constants (l=0,1,2), folded into M host-side
C0 = 0.28209479177387814
C1 = 0.4886025119029199
C2A = 1.0925484305920792
C2B = 0.31539156525252005
C2C = 0.5462742152960396
YLM_SCALE = np.array([C0, C1, C1, C1, C2A, C2A, C2B, C2A, C2C], dtype=np.float64)

_CACHE = {}


def _build_program():
    nc = bacc.Bacc("TRN2", target_bir_lowering=False, debug=False,
                   num_devices=NCORES)

    # ---- per-core DRAM I/O ----
    rpl = nc.dram_tensor("rpl", [128, 3 * T], F32, kind="ExternalInput").ap()
    m64d = nc.dram_tensor("m64d", [NQ, IJ], F32, kind="ExternalInput").ap()
    w1e2d = nc.dram_tensor("w1e2d", [W, 128], BF16, kind="ExternalInput").ap()
    w2e64d = nc.dram_tensor("w2e64d", [H, NQ], F32, kind="ExternalInput").ap()
    b1d = nc.dram_tensor("b1d", [H, 1], F32, kind="ExternalInput").ap()
    addvd = nc.dram_tensor("addvd", [NQ, 1], F32, kind="ExternalInput").ap()
    identd = nc.dram_tensor("identd", [128, 128], BF16, kind="ExternalInput").ap()
    out = nc.dram_tensor("out", [ZC, IJ], BF16, kind="ExternalOutput").ap()

    with tile.TileContext(nc) as tc:
        with tc.tile_pool(name="const", bufs=1) as cpool:
            # ---- resident constants (rpl first: everything downstream
            # of the monomial prologue waits on it) ----
            rpl_sb = cpool.tile([128, 3 * T], F32)
            nc.sync.dma_start(rpl_sb[:], rpl[:])
            m_stage = cpool.tile([NQ, IJ], F32)
            nc.sync.dma_start(m_stage[:], m64d[:])
            m_sb = cpool.tile([NQ, IJ], F32R)
            w1_sb = cpool.tile([W, 128], BF16)
            nc.sync.dma_start(w1_sb[:], w1e2d[:])
            w2_stage = cpool.tile([H, NQ], F32)
            nc.sync.dma_start(w2_stage[:], w2e64d[:])
            w2_sb = cpool.tile([H, NQ], F32R)
            b1_sb = cpool.tile([H, 1], F32)
            nc.sync.dma_start(b1_sb[:], b1d[:])
            addv_sb = cpool.tile([NQ, 1], F32)
            nc.sync.dma_start(addv_sb[:], addvd[:])
            id_sb = cpool.tile([128, 128], BF16)
            nc.sync.dma_start(id_sb[:], identd[:])

            x = rpl_sb[:, 0:T]
            y = rpl_sb[:, T:2 * T]
            z = rpl_sb[:, 2 * T:3 * T]

            # =========================================================
            # Prologue: channel planes.  chan col layout per tile t:
            #   t*64 + p*9 + l : ratio_l (replicated per path p)
            #   t*64 + 54 + l  : ratio_l (plain; l=0 is the ones channel)
            #   t*64 + 63      : radii
            # =========================================================
            chan = cpool.tile([128, W * T], BF16)
            aux = cpool.tile([128, 12 * T], F32)

            def ax(i):
                return aux[:, i * T:(i + 1) * T]

            (xx, yy, zz, r2, safe, inv2, invr, tmp,
             tmp2, tmp3, tmp4, tmp5) = (ax(i) for i in range(12))
            chv = chan[:].rearrange("p (t c) -> p t c", c=W)

            def pl(j):   # plain channel column j (one col per tile)
                return chv[:, :, j]

            nc.vector.tensor_tensor(xx, x, x, ALU.mult)
            nc.vector.tensor_tensor(yy, y, y, ALU.mult)
            nc.vector.tensor_tensor(zz, z, z, ALU.mult)
            nc.vector.tensor_tensor(tmp, xx, yy, ALU.add)
            nc.vector.tensor_tensor(r2, tmp, zz, ALU.add)
            # guard r2 == 0 exactly like the reference's safe_r2
            nc.vector.tensor_scalar(safe, r2, 0.0, None, ALU.is_equal)
            nc.vector.tensor_tensor(safe, r2, safe, ALU.add)
            nc.vector.reciprocal(inv2, safe)              # 1/safe_r2
            nc.scalar.sqrt(invr, inv2)                    # 1/safe_r (~7e-6 rel)

            # products that only need x/y/z go to Pool in parallel with
            # the DVE chain (GPSIMD has no PSUM port but SBUF ops are fine)
            nc.vector.memset(pl(54), 1.0)                 # ones (l=0)
            nc.vector.tensor_tensor(pl(55), y, invr, ALU.mult)
            nc.vector.tensor_tensor(pl(56), z, invr, ALU.mult)
            nc.vector.tensor_tensor(pl(57), x, invr, ALU.mult)
            nc.vector.tensor_tensor(tmp2, x, y, ALU.mult)
            nc.vector.tensor_tensor(pl(58), tmp2, inv2, ALU.mult)
            nc.vector.tensor_tensor(tmp3, y, z, ALU.mult)
            nc.vector.tensor_tensor(pl(59), tmp3, inv2, ALU.mult)
            nc.vector.scalar_tensor_tensor(tmp, zz, 3.0, r2, ALU.mult,
                                           ALU.subtract)
            nc.vector.tensor_tensor(pl(60), tmp, inv2, ALU.mult)
            nc.vector.tensor_tensor(tmp4, x, z, ALU.mult)
            nc.vector.tensor_tensor(pl(61), tmp4, inv2, ALU.mult)
            nc.vector.tensor_tensor(tmp5, xx, yy, ALU.subtract)
            nc.vector.tensor_tensor(pl(62), tmp5, inv2, ALU.mult)
            nc.vector.tensor_tensor(pl(63), r2, invr, ALU.mult)   # radii

            # replicate the 9 plain ratios into the 6 path blocks; the
            # first pair's 8 tiles go first so its transposes start while
            # the remaining 92 tiles replicate, and the one-time f32r
            # weight staging copies slot in between (off the critical path)
            chv8 = chan[:, 0:8 * W].rearrange("p (t c) -> p t c", c=W)
            chv16 = chan[:, 8 * W:16 * W].rearrange("p (t c) -> p t c", c=W)
            chv84 = chan[:, 16 * W:].rearrange("p (t c) -> p t c", c=W)
            for p in range(NPATH):
                nc.vector.tensor_copy(chv8[:, :, p * 9:p * 9 + 9],
                                      chv8[:, :, 54:63])
            nc.vector.tensor_copy(w2_sb[:], w2_stage[:])
            nc.vector.tensor_copy(m_sb[:], m_stage[:])
            for p in range(NPATH):
                nc.vector.tensor_copy(chv16[:, :, p * 9:p * 9 + 9],
                                      chv16[:, :, 54:63])

            # =========================================================
            # Main loop: 25 groups of 4 tiles; transposes batched in
            # pairs of groups (8 tiles -> [64, 1024] PSUM -> DMA bounce)
            # =========================================================
            NPAIR = (NG + 1) // 2     # 13 (last pair is half-size)

            with tc.tile_pool(name="shps", bufs=1, space="PSUM") as shpool, \
                 tc.tile_pool(name="hps", bufs=1, space="PSUM") as hpool, \
                 tc.tile_pool(name="kps", bufs=3, space="PSUM") as kpool, \
                 tc.tile_pool(name="tsb", bufs=2) as tspool, \
                 tc.tile_pool(name="work", bufs=2) as wpool, \
                 tc.tile_pool(name="kout", bufs=5) as opool:

                tp_ps = {}
                tps_sb = {}
                big_ps = {}

                def emit_pair(pr, half=None):
                    # transpose tiles of pair pr into PSUM (half=0: tiles
                    # 0-3, half=1: tiles 4-7, None: all).  The pair tile
                    # shares one physical PSUM bank with r_ps: the f32
                    # radial output lives on partitions 0-63 (matmul PSUM
                    # writes must start at partition 0), the bf16
                    # transposes on partitions 64-127 (transpose writes at
                    # base 64 pass the walrus ISA check).
                    t0 = pr * 2 * TB
                    ntile = min(2 * TB, T - t0)
                    if pr not in big_ps:
                        big = shpool.tile([128, 512], F32, tag="big",
                                          name="big_ps")
                        big_ps[pr] = big
                        tp_ps[pr] = big[NQ:128, :].bitcast(BF16)
                        tps_sb[pr] = tspool.tile([W, 2 * TB * 128], BF16,
                                                 tag="tsb", name="tps_sb")
                    tp = tp_ps[pr]
                    rng = range(ntile) if half is None else \
                        range(half * TB, min((half + 1) * TB, ntile))
                    for i in rng:
                        nc.tensor.transpose(
                            tp[:, i * 128:(i + 1) * 128],
                            chan[:, (t0 + i) * W:(t0 + i + 1) * W],
                            id_sb[:])

                def emit_tpscopy(g):
                    # bounce group g's tps half PSUM->SBUF (DVE 2x packed)
                    pr, hf = g // 2, g % 2
                    tsl = tps_sb[pr][:, hf * 512:hf * 512 + 512]
                    nc.vector.tensor_copy(
                        tsl, tp_ps[pr][:, hf * 512:hf * 512 + 512])
                    return tsl

                def emit_front(g, tsl):
                    # radial MLP hidden layer + relu
                    h_ps = hpool.tile([128, 512], F32, tag="hps")
                    nc.tensor.matmul(h_ps[:], w1_sb[:], tsl,
                                     start=True, stop=True)
                    h_sb = wpool.tile([128, 512], F32R, tag="hsb")
                    nc.scalar.activation(h_sb[:], h_ps[:], ACTF.Relu,
                                         bias=b1_sb[:])
                    return h_sb

                def emit_r(g, h_sb):
                    r_ps = big_ps[g // 2][0:NQ, :]
                    nc.tensor.matmul(r_ps, w2_sb[:], h_sb[:],
                                     start=True, stop=True)
                    return r_ps

                def emit_bmult(g, tsl, r_ps):
                    # fused B build: bstk = (R + addv) * Y  (one DVE op)
                    bstk = wpool.tile([NQ, 512], F32R, tag="bstk")
                    nc.vector.scalar_tensor_tensor(
                        bstk[:], r_ps, addv_sb[:], tsl,
                        ALU.add, ALU.mult)
                    return bstk

                def emit_main(g, bstk, dts):
                    # k=64 output matmuls for tiles `dts` of group g
                    tiles = []
                    for dt in dts:
                        k_ps = kpool.tile([128, IJ], F32, tag="kps")
                        for half in range(2):
                            nc.tensor.matmul(
                                k_ps[:, half * 512:(half + 1) * 512],
                                bstk[:, dt * 128:(dt + 1) * 128],
                                m_sb[:, half * 512:(half + 1) * 512],
                                start=True, stop=True)
                        tiles.append(k_ps)
                    return tiles

                def emit_copy(k_sb, dt, k_ps):
                    # PSUM f32 -> SBUF bf16; dt0 split across both engines
                    # so its k-PSUM slot frees early (dt2 reuses it)
                    dst = k_sb[:, dt * IJ:(dt + 1) * IJ]
                    if dt == 0:
                        nc.vector.tensor_copy(dst[:, 0:712], k_ps[:, 0:712])
                        nc.scalar.copy(dst[:, 712:IJ], k_ps[:, 712:IJ])
                    elif dt == 2:
                        nc.vector.tensor_copy(dst[:], k_ps[:])
                    else:
                        nc.scalar.copy(dst[:], k_ps[:])

                def emit_store(g, k_sb, hf):
                    # store half hf (2 tiles = 256 points) of group g; the
                    # first half is issued as soon as dt0/dt1 are copied so
                    # the DMA engines never sit idle waiting for dt3
                    z0 = 512 * g + hf * 256
                    if z0 + 256 <= ZC:
                        og = out[z0:z0 + 256, :].rearrange(
                            "(dt p) ij -> p dt ij", dt=2)
                        nc.sync.dma_start(
                            og, k_sb[:, hf * 2 * IJ:(hf + 1) * 2 * IJ]
                            .rearrange("p (dt ij) -> p dt ij", dt=2))
                    else:
                        for dt in (2 * hf, 2 * hf + 1):
                            zt = 512 * g + dt * 128
                            if zt >= ZC:
                                break
                            rows = min(128, ZC - zt)
                            nc.sync.dma_start(
                                out[zt:zt + rows, :],
                                k_sb[0:rows, dt * IJ:(dt + 1) * IJ])

                # Software-pipelined emission (v5 structure).  PE order:
                # [pair transposes (odd g)] [h_g] [main g-1: dt0,dt1] [R_g]
                # [main g-1: dt2,dt3]; the tps bounce for g heads the
                # iteration.
                emit_pair(0)
                prev = None      # (bstk) of group g-1
                tsl = emit_tpscopy(0)
                for g in range(NG):
                    nxt = emit_tpscopy(g + 1) if g + 1 < NG else None
                    h_sb = emit_front(g, tsl)
                    if g == 0:
                        # remaining channel replication (tiles 16-99) in
                        # iteration 0's otherwise-idle DVE window
                        for p in range(NPATH):
                            nc.vector.tensor_copy(
                                chv84[:, :, p * 9:p * 9 + 9],
                                chv84[:, :, 54:63])
                    if prev is not None:
                        pg, pb = prev
                        pk_sb = opool.tile([128, TB * IJ], BF16, tag="ksb")
                        t01 = emit_main(pg, pb, (0, 1))
                        emit_copy(pk_sb, 0, t01[0])
                        r_ps = emit_r(g, h_sb)
                        emit_copy(pk_sb, 1, t01[1])
                        emit_store(pg, pk_sb, 0)
                        t23 = emit_main(pg, pb, (2, 3))
                        emit_copy(pk_sb, 2, t23[0])
                        emit_copy(pk_sb, 3, t23[1])
                        emit_store(pg, pk_sb, 1)
                    else:
                        r_ps = emit_r(g, h_sb)
                    prev = (g, emit_bmult(g, tsl, r_ps))
                    if g + 2 < NG:
                        emit_pair((g + 2) // 2, (g + 2) % 2)
                    tsl = nxt
                pg, pb = prev
                pk_sb = opool.tile([128, TB * IJ], BF16, tag="ksb")
                # the final group only has valid points in its first tiles;
                # skip matmuls/copies/stores for pure-padding tiles
                ndt = min(TB, -(-(ZC - 512 * pg) // 128))
                for dt, kt in zip(range(ndt), emit_main(pg, pb, tuple(range(ndt)))):
                    emit_copy(pk_sb, dt, kt)
                emit_store(pg, pk_sb, 0)
                if ndt > 2:
                    emit_store(pg, pk_sb, 1)

    nc.compile()
    return nc


def _get_program():
    if "nc" not in _CACHE:
        _CACHE["nc"] = _build_program()
    return _CACHE["nc"]


def _host_prep(r, W1, b1, W2, b2, cg, ylm_mix, rf_mix, norm_coef):
    r = np.asarray(r, dtype=np.float32)
    W1 = np.asarray(W1, dtype=np.float32)
    b1 = np.asarray(b1, dtype=np.float32)
    W2 = np.asarray(W2, dtype=np.float32)
    b2 = np.asarray(b2, dtype=np.float32)
    cg = np.asarray(cg, dtype=np.float32)
    ylm_mix = np.asarray(ylm_mix, dtype=np.float32)
    rf_mix = np.asarray(rf_mix, dtype=np.float32)
    norm_coef = np.asarray(norm_coef, dtype=np.float32)

    # constant folding: M64[p*9+l, ij] = sum_k rf[k,p] ylm_s[k,l] cg[k,ij] * nc0
    ylm_s = ylm_mix.astype(np.float64) * YLM_SCALE[None, :]
    wkp = rf_mix.astype(np.float64)[:, :, None] * ylm_s[:, None, :]  # [K,p,l]
    mcore = np.einsum("kq,kj->qj", wkp.reshape(KDIM, 54),
                      cg.astype(np.float64).reshape(KDIM, IJ))
    nc0 = norm_coef.astype(np.float64)[:, :, 0].reshape(1, IJ)
    m64 = np.zeros((NQ, IJ), dtype=np.float32)
    m64[0:54] = (mcore * nc0).astype(np.float32)

    import ml_dtypes
    w1e2 = np.zeros((W, 128), dtype=ml_dtypes.bfloat16)
    w1e2[63, :] = W1[0].astype(ml_dtypes.bfloat16)

    w2e64 = np.zeros((H, NQ), dtype=np.float32)
    w2e64[:, 0:54] = np.repeat(W2, 9, axis=1)

    addv = np.zeros((NQ, 1), dtype=np.float32)
    addv[0:54, 0] = np.repeat(b2, 9)
    addv[54:63, 0] = 1.0

    shared = {
        "m64d": m64,
        "w1e2d": w1e2,
        "w2e64d": w2e64,
        "b1d": np.ascontiguousarray(b1.reshape(H, 1)),
        "addvd": addv,
        "identd": np.eye(128, dtype=ml_dtypes.bfloat16),
    }

    in_maps = []
    for c in range(NCORES):
        rs = r[c * ZC:(c + 1) * ZC]
        rp = np.empty((ZC_PAD, 3), dtype=np.float32)
        rp[:ZC] = rs
        rp[ZC:] = np.array([1.0, 0.0, 0.0], dtype=np.float32)
        rpl = rp.reshape(T, 128, 3).transpose(1, 2, 0).reshape(128, 3 * T)
        m = dict(shared)
        m["rpl"] = np.ascontiguousarray(rpl)
        in_maps.append(m)
    return in_maps


def _run_device(in_maps, trace=False, **kw):
    nc = _get_program()
    return run_bass_kernel_spmd(nc, in_maps, core_ids=list(range(NCORES)),
                                trace=trace, **kw)


def kernel(r, W1, b1, W2, b2, cg, ylm_mix, rf_mix, norm_coef):
    r = np.asarray(r, dtype=np.float32)
    norm_coef_f = np.asarray(norm_coef, dtype=np.float32)
    in_maps = _host_prep(r, W1, b1, W2, b2, cg, ylm_mix, rf_mix, norm_coef_f)
    res = _run_device(in_maps)
    out = np.concatenate(
        [np.asarray(res.results[c]["out"]).astype(np.float32)
         for c in range(NCORES)], axis=0)

    # points with exactly zero radius use norm_coef[..., 1] instead of [..., 0]
    x, y, z = r[:, 0], r[:, 1], r[:, 2]
    r2 = (x * x + y * y) + z * z
    zero = r2 == np.float32(0.0)
    if np.any(zero):
        scale = (norm_coef_f[:, :, 1].astype(np.float64)
                 / norm_coef_f[:, :, 0].astype(np.float64)).reshape(1, IJ)
        out[zero] = (out[zero].astype(np.float64) * scale).astype(np.float32)

    return out.reshape(Z, DO, DI)
